# revision 1
# baseline (speedup 1.0000x reference)
"""AdaptiveAngleConv Trainium2 kernel.

Computes, for 4 rotated variants of a 3x3 kernel, y[a] = conv2d(x, rot_a(W)) + b
  x: [16, 64, 128, 128] f32, W: [64, 64, 3, 3] f32, b: [64, 1, 1] f32
  out: [4, 16, 64, 128, 128] f32

Strategy: pure data-parallel over batch (2 images per core, 8 cores, no
collectives). Each core runs an implicit-GEMM conv over 4-row output strips
(N=512 = one f32 PSUM bank), with the 4 angle variants merged into the
matmul M dimension as two angle-pairs (M = 2 angles x 64 Cout = 128).

Per strip, 9 matmuls (the packing floor) instead of a naive 36:
 - x is staged in SBUF FOUR times as two dual-copy planes of one tile
   xt[128, 2, HP*WP]:
     plane 0 (A): partitions 0-63 = x, partitions 64-127 = x advanced 2 COLS
     plane 1 (B): partitions 0-63 = x, partitions 64-127 = x advanced 2 ROWS
   Plane A fuses the 3 horizontal tap pairs (kh,0)+(kh,2) into K=128 matmuls;
   plane B fuses the vertical pair (0,1)+(2,1) into a 4th K=128 matmul.
   8 non-center taps -> 4 full-K matmuls per angle-pair: a perfect matching
   of the 3x3 tap grid minus center (3 horizontal deltas + 1 vertical delta).
 - The center tap is rotation-invariant (PERMS[:,4]==4 for all angles), so
   its contribution is computed once per strip (one K=64 matmul, [C;C]
   layout) and added during eviction: st = psum + (center + bias) on the
   VectorE.

The four shifted copies are materialized ON THE HOST into one DRAM tensor
xq[b][128, 2, HP*WP], so each x chunk load is ONE or TWO DMA instructions
(DGE descriptor-generation latency, ~625-1040 ns serial per queue, is the
head bottleneck — not bytes). All x loads ride the sync/HWDGE queue so the
serial DMA device receives chunks in need-order (mixing queues lets a later
chunk overtake an earlier one and head-of-line-stall the in-order PE queue);
stores split across HWDGE + SWDGE. Strips are processed one at a time
(center then both angle-pairs) so a chunk needed by strip s+1 never blocks
strip s's matmuls. Chunks are issued LOOKROWS output rows ahead of first
use. Weights are pre-permuted on the host into matmul lhsT slot layout and
split three ways (ap0+center ride ahead of ap1, interleaved with chunk 0 per
HEAD_PLAN). Everything TensorEngine-facing is bf16 (rel-err ~2.9e-3 inc.
bf16 output staging, well under the 2e-2 gate); accumulation is f32 in PSUM.
Junk warm-up matmuls ramp the PE p-state during the input DMA head. The
final group stores per-strip, the last strip as one fused two-angle store
per ap, to minimize the drain chain.

TimelineSim: 131.0 us = 122.9 us PE floor (576 matmuls x 213.3 ns, gapless
and at full p-state throughout) + ~4.1 us head (preamble barrier + DGE
pipeline + first chunk + sem prop) + ~4.0 us tail (last evict + store DGE +
transfer + sem prop + exit barrier) — both ends at their fixed-cost floors.
DMA device ~95.5 us busy (73%), DVE ~84.6 us, Pool ~67 us, ACT ~39 us.
Prior baselines: 158.7 us (11-matmul packing), 133.2 us (9-matmul packing
with 4 separate per-copy chunk loads).
"""

import numpy as np
import ml_dtypes

import concourse.bass as bass
import concourse.mybir as mybir
from concourse import tile

PERMS = np.array(
    [
        [0, 1, 2, 3, 4, 5, 6, 7, 8],
        [3, 0, 1, 6, 4, 2, 7, 8, 5],
        [6, 3, 0, 7, 4, 1, 8, 5, 2],
        [7, 6, 3, 8, 4, 0, 5, 2, 1],
    ],
    dtype=np.int32,
)

N_CORES = 8
B, CIN, COUT, H, W = 16, 64, 64, 128, 128
BPC = B // N_CORES  # batch images per core
HP, WP = H + 2, W + 2  # padded
STRIP = 4  # output rows per PSUM bank fill
NFREE = STRIP * W  # 512
NSLOT = 9  # lhsT weight slots

# tunables (module-level so perf sweeps can override before build_nc)
CHUNKS = [(0, 6), (6, 14)] + [(lo, min(lo + 8, 130)) for lo in range(14, 130, 8)]
LOOKROWS = 24  # issue a chunk this many output rows before first use
NJUNK = 12
JUNK_COLS = 256
# head DMA issue plan: (queue 's'=sync/HWDGE | 'g'=gpsimd/SWDGE, item)
HEAD_PLAN = [
    ("s", "A0"), ("s", "wt_ap0"), ("s", "B0"),
    ("g", "wt_c"), ("g", "wt_ap1"), ("g", "bias"),
]
# very last strip: ap0 stores whole on LAST_AP0_Q; ap1 is computed/evicted/
# stored in row-pieces (sum = STRIP). [4] (no split) measured best: finer
# pieces lose more to serialized DGE gens + DVE evictions than they save in
# final-transfer size. One queue char per ap1 piece.
LAST_SPLIT = [4]
LAST_AP0_Q = "s"
LAST_Q = "s"
# if set, the very last eviction is split DVE [0:h] || Pool [h:], running the
# two halves concurrently so the final store's wait clears sooner
LAST_EVICT_SPLIT = None


def _split_multiwait_ctrl(nc, end_times=None):
    """This container's walrus encodes at most one sync-wait per instruction
    (Drain/Matmult/... all hit 'Too many sync wait commands' with >1). Move
    extra waits onto single-wait NOPs preceding the instruction on the same
    engine.

    Multi-waits are ordered so the latest-completing sem stays on the real
    instruction: earlier NoOp waits then retire during its stall window
    instead of serializing after it. Completion-time key: `end_times` (a
    {instruction_name: simulated end ns} map from a prior TimelineSim pass)
    when given, else the program position of the sem's last updater.
    """
    nsplit = 0
    for f in nc.m.functions:
        upd = {}
        order = {}
        idx = 0
        for blk in f.blocks:
            for inst in blk.instructions:
                idx += 1
                order[inst.name] = idx
                s2 = inst.sync_info
                if s2 is not None:
                    for u in s2.on_update:
                        upd.setdefault(u.ant_name, []).append((idx, inst.name))

        def sort_key(iname):
            def k(w):
                us = upd.get(w.ant_name, [])
                if not us:
                    return 0.0
                if end_times:
                    # relevant updater: last one preceding this instruction
                    my = order.get(iname, 1 << 30)
                    prev = [n for (i, n) in us if i < my]
                    target = prev[-1] if prev else us[-1][1]
                    et = end_times.get(target)
                    if et is not None:
                        return et
                return float(us[-1][0])
            return k

        for blk in f.blocks:
            newlist = []
            for inst in blk.instructions:
                si = inst.sync_info
                if si is not None and len(si.on_wait) > 1:
                    waits = list(si.on_wait)
                    if all(w.wait_mode == "sem-ge-imm" for w in waits):
                        # safe to reorder: >= waits are monotonic
                        waits.sort(key=sort_key(inst.name))
                    for w in waits[:-1]:
                        d = mybir.InstNoOp(
                            name=f"{inst.name}-wsplit{nsplit}", ins=[], outs=[]
                        )
                        nsplit += 1
                        d.engine = inst.engine
                        d.sync_info = mybir.SyncInfo(on_wait=[w], on_update=[])
                        newlist.append(d)
                    si.on_wait = [waits[-1]]
                newlist.append(inst)
            blk.instructions = newlist
    return nsplit


def _sim_instruction_end_times(nc):
    """Run TimelineSim capturing each instruction's engine-span end time.
    Used to drive the timing-informed multiwait sort (second build pass)."""
    import concourse.timeline_sim as tsim

    class _Rec:
        def __init__(self):
            self.end = {}
        def enable_explicit_ordering(self, *a, **k): pass
        def reserve_process_order(self, *a, **k): pass
        def add_event(self, process, thread, name, ts, dur=None, unit="s",
                      args=None, clock_name=None, flows=None,
                      terminating_flows=None):
            if args and dur not in (None, "NO_END"):
                n = args.get("instruction_name")
                if n:
                    self.end[n] = max(self.end.get(n, 0.0), ts + dur)
        def add_end(self, *a, **k): pass
        def __getattr__(self, name):
            return lambda *a, **kw: None

    rec = _Rec()
    orig = tsim._build_perfetto
    tsim._build_perfetto = lambda core_id: rec
    try:
        tsim.TimelineSim(nc, trace=True).simulate()
    finally:
        tsim._build_perfetto = orig
    return rec.end


def build_nc(n_batch=BPC, split_ctrl=True, loop_r=None, _end_times=None):
    """loop_r: wrap the whole compute in a For_i repeating it loop_r times —
    used only for on-hardware timing (wall-clock delta between two loop_r
    values divided by the iteration delta isolates per-iteration HW time).

    With split_ctrl, builds twice: the first (position-proxy multiwait sort)
    is simulated to harvest per-instruction end times, which drive a
    timing-informed sort in the second build. Falls back to the proxy build
    if the refinement pass fails for any reason."""
    if split_ctrl and _end_times is None:
        nc = build_nc(n_batch, split_ctrl, loop_r, _end_times={})
        try:
            et = _sim_instruction_end_times(nc)
            if et:
                nc2 = build_nc(n_batch, split_ctrl, loop_r, _end_times=et)
                return nc2
        except Exception:
            pass
        return nc
    bf16 = mybir.dt.bfloat16
    f32 = mybir.dt.float32
    nc = bass.Bass(target_bir_lowering=False)
    # Strip dead framework preamble work that gates the entry barrier:
    #  - four memsets of const-* scalar tiles this kernel never reads (BIR
    #    verifier: "no reader" for all four)
    #  - the per-engine zero/bcreg/monotonic RegisterMoves: no instruction
    #    in this program references any of those registers (verified by
    #    operand scan; the kernel has no branches/compares/monotonic sems)
    # Dropping them releases the all-engine barrier ~0.7 us earlier.
    import re as _re
    for _f in nc.m.functions:
        for _blk in _f.blocks:
            _blk.instructions = [
                _i for _i in _blk.instructions
                if not (
                    type(_i).__name__ == "InstMemset"
                    and str(_i.engine).endswith("Pool")
                    and _i.outs
                    and "const-" in str(_i.outs[0])
                    and list(map(list, _i.outs[0].ap)) == [[1, 128], [1, 1]]
                )
                and not (
                    type(_i).__name__ == "InstRegisterMove"
                    and _i.outs
                    and _re.search(
                        r"regref='[A-Za-z]+_(bcreg|zero|monotonic)", str(_i.outs[0])
                    )
                )
            ]
    xq_d = nc.declare_dram_parameter(
        "xq", [n_batch, 128, 2, HP * WP], bf16, isOutput=False
    )
    wt_d = nc.declare_dram_parameter("wt", [128, NSLOT * 128], bf16, isOutput=False)
    bias_d = nc.declare_dram_parameter("bias2", [128, 1], f32, isOutput=False)
    out_d = nc.declare_dram_parameter(
        "out", [4, n_batch, COUT, H, W], bf16, isOutput=True
    )

    with tile.TileContext(nc) as tc:
        with (
            tc.tile_pool(name="const", bufs=1) as const_pool,
            tc.tile_pool(name="xpool", bufs=2) as xpool,
            tc.tile_pool(name="psum", bufs=8, space="PSUM") as psum_pool,
            tc.tile_pool(name="stage", bufs=8) as stage_pool,
        ):
            import contextlib

            loop_ctx = tc.For_i(0, loop_r, 1) if loop_r else contextlib.nullcontext()
            with loop_ctx:
                body(nc, const_pool, xpool, psum_pool, stage_pool,
                     xq_d, wt_d, bias_d, out_d, n_batch,
                     use_swdge=loop_r is None)
    if split_ctrl:
        _split_multiwait_ctrl(nc, end_times=_end_times or None)
    return nc


def body(nc, const_pool, xpool, psum_pool, stage_pool, xq_d, wt_d, bias_d, out_d, n_batch, use_swdge=True):
    gpeng = nc.gpsimd if use_swdge else nc.sync
    bf16 = mybir.dt.bfloat16
    f32 = mybir.dt.float32
    GROUP = 2
    if True:
        if True:
            # PE pre-warm: junk matmuls on a zeroed tile ramp the PE p-state
            # while the first x chunk is still in flight. Issued before any
            # DMA so the scheduler gives them the earliest PE priority (a
            # hoisted real Ldweights would head-of-line-block the PE queue
            # on the wt DMA otherwise).
            junk_sb = const_pool.tile([128, max(JUNK_COLS, 128)], bf16)
            nc.vector.memset(junk_sb[:], 0)
            for w in range(NJUNK):
                jps = psum_pool.tile([128, JUNK_COLS], f32, tag="ps", name=f"jps{w}")
                nc.tensor.matmul(jps[:], junk_sb[:, 0:128], junk_sb[:, 0:JUNK_COLS])

            # wt is loaded per the HEAD_PLAN below (split so early matmuls
            # aren't gated on weight slots they don't need yet)
            wt_sb = const_pool.tile([128, NSLOT * 128], bf16)
            bias_sb = const_pool.tile([128, 1], f32)

            # xt planes: [:, 0] = A = [x | x+2cols], [:, 1] = B = [x | x+2rows]
            xtiles = [
                xpool.tile([128, 2, HP * WP], bf16, tag="xt", name=f"xt{b}")
                for b in range(n_batch)
            ]

            def load_chunk(b, lo, hi):
                # ALL x loads ride the sync (HWDGE) queue, split into A-plane
                # then B-plane DMAs: per-queue FIFO DGE keeps the serial DMA
                # device processing chunks in need-order (mixing queues lets a
                # later sync chunk overtake an earlier gpsimd one, starving
                # the in-order PE queue)
                xt = xtiles[b]
                nc.sync.dma_start(
                    xt[:, 0, lo * WP : hi * WP], xq_d[b][:, 0, lo * WP : hi * WP]
                )
                nc.sync.dma_start(
                    xt[:, 1, lo * WP : hi * WP], xq_d[b][:, 1, lo * WP : hi * WP]
                )

            # pending chunk loads, issued interleaved with strips. A chunk
            # (b2, lo, hi) is first needed by strip r0 = lo-5 of image b2
            # (strip windows read padded rows <= r0+5); issue it LOOKROWS of
            # absolute output rows ahead of that so its transfer lands before
            # any PE-queue wait parks on it (in-order SEQ: a late chunk for
            # strip s head-of-line blocks strips < s too).
            pending = [(b, lo, hi) for b in range(n_batch) for (lo, hi) in CHUNKS]
            # head plan: ordered (queue, item) issue list for the first-strip
            # dependencies. Items: wt_c (center slot), wt_ap0, wt_ap1, A0/B0
            # (chunk-0 planes), bias. The DGE pipeline (~625 ns/DMA + 650 ns
            # start latency, serial per queue) paces the head, so order and
            # queue assignment are swept empirically.
            _, lo0, hi0 = pending.pop(0)
            xt0 = xtiles[0]
            items = {
                "wt_c": (wt_sb[:, 512:640], wt_d[:, 512:640]),
                "wt_ap0": (wt_sb[:, 0:512], wt_d[:, 0:512]),
                "wt_ap1": (wt_sb[:, 640:1152], wt_d[:, 640:1152]),
                "A0": (xt0[:, 0, lo0 * WP : hi0 * WP], xq_d[0][:, 0, lo0 * WP : hi0 * WP]),
                "B0": (xt0[:, 1, lo0 * WP : hi0 * WP], xq_d[0][:, 1, lo0 * WP : hi0 * WP]),
                "bias": (bias_sb[:], bias_d[:]),
            }
            for q, it in HEAD_PLAN:
                eng = nc.sync if q == "s" else gpeng
                dst, src = items.pop(it)
                eng.dma_start(dst, src)
            assert not items, f"HEAD_PLAN missed {list(items)}"

            def issue_ready(b, r0):
                cur = b * H + r0
                while pending:
                    b2, lo, hi = pending[0]
                    if b2 * H + max(lo - 5, 0) <= cur + LOOKROWS:
                        load_chunk(*pending.pop(0))
                    else:
                        break

            def do_strip(b, xvA, xvB, r0, nrows, sts, st_col):
                """Center + both angle-pairs for output rows [r0, r0+nrows).
                Evictions land at st_col of the per-ap staging tiles."""
                nfree = nrows * W
                cps = psum_pool.tile([128, nfree], f32, tag="ps", name=f"cps{b}_{r0}")
                nc.tensor.matmul(
                    cps[:],
                    wt_sb[0:64, 4 * 128 : 5 * 128],
                    xvA[0:64, r0 + 1 : r0 + 1 + nrows, 1 : 1 + W],
                )
                c2sb = stage_pool.tile([128, nfree], f32, tag="c2", name=f"c2_{b}_{r0}")
                nc.scalar.activation(
                    c2sb[:],
                    cps[:],
                    mybir.ActivationFunctionType.Identity,
                    bias=bias_sb[:],
                )
                for ap in range(2):
                    ps = psum_pool.tile([128, nfree], f32, tag="ps")
                    base = 0 if ap == 0 else 5
                    # K=128 pairs on plane A: taps (kh,0) lower + (kh,2) upper
                    for j in range(3):
                        s = base + j
                        nc.tensor.matmul(
                            ps[:],
                            wt_sb[:, s * 128 : (s + 1) * 128],
                            xvA[:, r0 + j : r0 + j + nrows, 0:W],
                            start=(j == 0),
                            stop=False,
                        )
                    # K=128 pair on plane B: taps (0,1) lower + (2,1) upper
                    nc.tensor.matmul(
                        ps[:],
                        wt_sb[:, (base + 3) * 128 : (base + 4) * 128],
                        xvB[:, r0 : r0 + nrows, 1 : 1 + W],
                        start=False,
                        stop=True,
                    )
                    # eviction: st = ps + (center + bias), DVE only
                    nc.vector.tensor_add(
                        sts[ap][:, st_col : st_col + nfree], ps[:], c2sb[:]
                    )

            n_groups = H // (STRIP * GROUP)
            for b in range(n_batch):
                xvA = xtiles[b][:, 0].rearrange("p (h w) -> p h w", w=WP)
                xvB = xtiles[b][:, 1].rearrange("p (h w) -> p h w", w=WP)

                for g in range(n_groups):
                    rg = g * GROUP * STRIP  # first output row of the group
                    last_group = b == n_batch - 1 and g == n_groups - 1
                    if not last_group:
                        # per-ap staging tiles spanning the whole group
                        # (stored once per ap at group end)
                        sts = [
                            stage_pool.tile(
                                [128, GROUP * NFREE], bf16, tag="st",
                                name=f"st{b}_{g}_{ap}",
                            )
                            for ap in range(2)
                        ]
                        # strips are fully processed one at a time (center,
                        # then both angle-pairs) so a DMA chunk needed by
                        # strip si+1 never head-of-line blocks strip si's
                        # matmuls on the in-order PE queue.
                        for si in range(GROUP):
                            r0 = rg + si * STRIP
                            issue_ready(b, r0)
                            do_strip(b, xvA, xvB, r0, STRIP, sts, si * NFREE)
                        for ap in range(2):
                            for al in range(2):
                                a = 2 * ap + al
                                eng = nc.sync if al == 0 else gpeng
                                eng.dma_start(
                                    out_d[a, b, :, rg : rg + GROUP * STRIP, :],
                                    sts[ap][al * 64 : (al + 1) * 64, :],
                                )
                    else:
                        # final group: per-strip stores so the first strip's
                        # transfers overlap the last strip's matmuls; the last
                        # strip uses ONE fused two-angle store per ap (single
                        # issue chain ends earlier than staggered transfers)
                        for si in range(GROUP):
                            r0 = rg + si * STRIP
                            issue_ready(b, r0)
                            stk = stage_pool.tile(
                                [128, 2, NFREE], bf16, tag="stz", bufs=2,
                                name=f"stz{si}",
                            )
                            if si < GROUP - 1:
                                stv = [stk[:, ap] for ap in range(2)]
                                do_strip(b, xvA, xvB, r0, STRIP, stv, 0)
                                for ap in range(2):
                                    for al in range(2):
                                        a = 2 * ap + al
                                        eng = nc.sync if al == 0 else gpeng
                                        eng.dma_start(
                                            out_d[a, b, :, r0 : r0 + STRIP, :],
                                            stk[al * 64 : (al + 1) * 64, ap],
                                        )
                                continue
                            # very last strip: shared center (full strip),
                            # whole ap0, then ap1 in LAST_SPLIT row-pieces
                            q = {"s": nc.sync, "g": gpeng}
                            cps = psum_pool.tile([128, NFREE], f32, tag="ps", name="cpsL")
                            nc.tensor.matmul(
                                cps[:],
                                wt_sb[0:64, 4 * 128 : 5 * 128],
                                xvA[0:64, r0 + 1 : r0 + 1 + STRIP, 1 : 1 + W],
                            )
                            c2sb = stage_pool.tile([128, NFREE], f32, tag="c2", name="c2L")
                            nc.scalar.activation(
                                c2sb[:], cps[:],
                                mybir.ActivationFunctionType.Identity,
                                bias=bias_sb[:],
                            )
                            pieces = [(0, STRIP, 0)] + [
                                (sum(LAST_SPLIT[:k]), nr, 1)
                                for k, nr in enumerate(LAST_SPLIT)
                            ]
                            for pi, (ro, nr, ap) in enumerate(pieces):
                                ps = psum_pool.tile([128, nr * W], f32, tag="ps")
                                base = 0 if ap == 0 else 5
                                for j in range(3):
                                    s = base + j
                                    nc.tensor.matmul(
                                        ps[:],
                                        wt_sb[:, s * 128 : (s + 1) * 128],
                                        xvA[:, r0 + ro + j : r0 + ro + j + nr, 0:W],
                                        start=(j == 0),
                                        stop=False,
                                    )
                                nc.tensor.matmul(
                                    ps[:],
                                    wt_sb[:, (base + 3) * 128 : (base + 4) * 128],
                                    xvB[:, r0 + ro : r0 + ro + nr, 1 : 1 + W],
                                    start=False,
                                    stop=True,
                                )
                                sl = stk[:, ap, ro * W : (ro + nr) * W]
                                c2s = c2sb[:, ro * W : (ro + nr) * W]
                                if pi == len(pieces) - 1 and LAST_EVICT_SPLIT:
                                    h = LAST_EVICT_SPLIT
                                    nc.vector.tensor_add(
                                        sl[:, 0:h], ps[:, 0:h], c2s[:, 0:h]
                                    )
                                    nc.gpsimd.tensor_add(
                                        sl[:, h:], ps[:, h:], c2s[:, h:]
                                    )
                                else:
                                    nc.vector.tensor_add(sl, ps[:], c2s)
                                if ap == 0:
                                    eng = q[LAST_AP0_Q]
                                else:
                                    eng = q[LAST_Q[pi - 1] if pi - 1 < len(LAST_Q) else LAST_Q[-1]]
                                eng.dma_start(
                                    out_d[2 * ap : 2 * ap + 2, b, :, r0 + ro : r0 + ro + nr, :],
                                    sl,
                                )


def prep_weights(weight, bias):
    """wt: [128, 9*128] bf16 lhsT layout; bias2: [128, 1] f32.

    Slots 0-3: angle-pair 0, slot 4: shared center, slots 5-8: angle-pair 1
    (center rides the first wt DMA half together with ap0). Per ap: slots
    +0..+2 are K=128 pairs {tap (kh,0) lower | tap (kh,2) upper} read from
    plane A; slot +3 is the K=128 pair {tap (0,1) lower | tap (2,1) upper}
    read from plane B (upper copy advanced 2 rows). The center tap (flat 4)
    is rotation-invariant (PERMS[:,4]==4), computed once per strip.
    """
    wflat = np.asarray(weight, np.float32).reshape(COUT, CIN, 9)
    # L[t][c, a, o] = wflat[o, c, PERMS[a, t]]
    L = wflat[:, :, PERMS].transpose(3, 1, 2, 0)  # [9, c, a, o]
    wt = np.zeros((128, NSLOT, 128), np.float32)
    for ap in range(2):
        base = 0 if ap == 0 else 5
        La = L[:, :, 2 * ap : 2 * ap + 2, :].reshape(9, CIN, 128)  # [t, c, m]
        for j in range(3):
            wt[0:64, base + j] = La[3 * j + 0]  # tap (j, 0) lower
            wt[64:128, base + j] = La[3 * j + 2]  # tap (j, 2) upper
        wt[0:64, base + 3] = La[1]  # tap (0, 1) lower (plane B lower = x)
        wt[64:128, base + 3] = La[7]  # tap (2, 1) upper (plane B upper = x+2rows)
    # shared center at slot 4 (so it rides the fast HWDGE wt half):
    # lhsT[c, al*64+o] = W[o, c, 4] duplicated for both angles
    w4 = wflat[:, :, 4].T  # [c, o]
    wt[0:64, 4] = np.concatenate([w4, w4], axis=1)
    wt = wt.reshape(128, NSLOT * 128).astype(ml_dtypes.bfloat16)
    bias2 = np.tile(np.asarray(bias, np.float32).reshape(COUT), 2)[:, None]
    return wt, np.ascontiguousarray(bias2, np.float32)


def prep_x(x):
    """Build the 4-copy SBUF staging layout on the host.

    Returns xq [B, 128, 2, HP*WP] bf16:
      xq[b, 0:64,   0] = x padded (plane A lower)
      xq[b, 64:128, 0] = x padded, advanced 2 columns (plane A upper)
      xq[b, 0:64,   1] = x padded (plane B lower)
      xq[b, 64:128, 1] = x padded, advanced 2 rows (plane B upper)
    """
    nb = x.shape[0]
    xp = np.zeros((nb, CIN, HP, WP), np.float32)
    xp[:, :, 1 : H + 1, 1 : W + 1] = np.asarray(x, np.float32)
    flat = xp.reshape(nb, CIN, HP * WP).astype(ml_dtypes.bfloat16)
    xq = np.zeros((nb, 128, 2, HP * WP), ml_dtypes.bfloat16)
    xq[:, 0:64, 0] = flat
    xq[:, 0:64, 1] = flat
    xq[:, 64:128, 0, : HP * WP - 2] = flat[:, :, 2:]
    xq[:, 64:128, 1, : HP * WP - 2 * WP] = flat[:, :, 2 * WP :]
    return xq


_CACHE = {}


def _enable_persistent_compile_cache():
    # NEFF compiles take 1-7 minutes; jax's persistent cache serializes the
    # compiled executable (NEFF included) so fresh processes skip the
    # recompile. Best-effort: ignored if the PJRT backend can't serialize.
    try:
        import jax

        jax.config.update("jax_compilation_cache_dir", "/tmp/jax_comp_cache")
        jax.config.update("jax_persistent_cache_min_compile_time_secs", 1.0)
    except Exception:
        pass


def kernel(x, weight, bias):
    from concourse import bass2jax as b2j

    _enable_persistent_compile_cache()

    x = np.asarray(x)
    in_dtype = x.dtype
    xq = prep_x(x)  # [B, 128, 2, HP*WP] bf16
    wt, bias2 = prep_weights(weight, bias)

    if "nc" not in _CACHE:
        _CACHE["nc"] = build_nc()
    nc = _CACHE["nc"]
    in_maps = [
        {"xq": xq[i * BPC : (i + 1) * BPC], "wt": wt, "bias2": bias2}
        for i in range(N_CORES)
    ]
    results = b2j.run_bass_via_pjrt(nc, in_maps, n_cores=N_CORES)
    out = np.stack([r["out"] for r in results])  # [N_CORES, 4, BPC, ...]
    out = out.transpose(1, 0, 2, 3, 4, 5).reshape(4, B, COUT, H, W)
    return out.astype(in_dtype)



# revision 8
# speedup vs baseline: 1.3044x; 1.3044x over previous
"""AdaptiveAngleConv Trainium2 kernel (error-corrected fp8 DoubleRow edition).

Computes, for 4 rotated variants of a 3x3 kernel, y[a] = conv2d(x, rot_a(W)) + b
  x: [16, 64, 128, 128] f32, W: [64, 64, 3, 3] f32, b: [64, 1, 1] f32
  out: [4, 16, 64, 128, 128] f32

Strategy: pure data-parallel over batch (2 images per core, 8 cores, no
collectives). Each core runs an implicit-GEMM conv over 4-row output strips
(N=512 = one f32 PSUM bank), with the 4 angle variants merged into the
matmul M dimension as two angle-pairs (M = 2 angles x 64 Cout = 128).

All matmuls are fp8e4 (e4m3) in DoubleRow perf mode: 2 K=128 subtiles per
matmul (K_eff=256) at 0.5 cycles per output row — 2x bf16 MAC throughput.
Plain e4m3 is far too noisy (4.2% rms per operand), so operands are
error-corrected: both x and W are split into e4m3 hi + e4m3 residual (lo)
planes, and the three significant cross products hi*hi + hi*lo + lo*hi are
accumulated (the dropped lo*lo term is ~2e-3 relative). Per angle-pair per
strip that is exactly 12 K=128 subtile slots = 6 DoubleRow matmuls
(1536 cycles vs 2048+ for bf16):
  M1-M3: pair {tap(kh,0) | tap(kh,2)} (partition-dual A-plane), subtiles
         {x_hi, x_lo} with W_hi               -> hi*hi + hi*lo, kh = 0,1,2
  M4:    pair {tap(0,1) | tap(2,1)} (B-plane), subtiles {x_hi, x_lo} w/ V_hi
  M5:    subtiles {A_hi rows r0, A_hi rows r0+1} with {Wp_lo(0), Wp_lo(1)}
  M6:    subtiles {A_hi rows r0+2, B_hi rows r0} with {Wp_lo(2), Wv_lo}
M5/M6 use custom overlapping access patterns (subtile stride = W or
RSZ-2W within the fused x tile). The rotation-invariant center tap is one
shared DoubleRow matmul (subtiles {x_hi, x_lo} w/ W4_hi; its W4_lo term is
dropped) evicted via ACT (+bias) and added to both angle-pairs on the DVE.
Measured end-to-end error: ~9e-3 (gate 2e-2); PE floor 13 matmuls x 106.7 ns
x 64 strips = 88.8 us.

x is staged in SBUF as ONE fused fp8 tile per image xt[128, 4, 129*128]
with regions [A-lo | A-hi | B-hi | B-lo] (A: tight rows, partitions =
[x | x+2cols]; B: [x+1col | x+2rows+1col]), ordered so every DoubleRow
subtile pair is a positive stride under the 32767-element matmul ifmap ISA
limit, host-prepped in DRAM so each chunk load is ONE DMA. With PE at 88.8 us the
kernel is DMA-device-bound: ~34.2 MB total traffic / 360 GB/s ~ 95 us serial
transfer + DGE head + store tail. All x loads ride the sync/HWDGE queue in
need-order; stores split across HWDGE + SWDGE (see baseline notes: mixing
queues for loads lets late chunks overtake and head-of-line-stall the
in-order PE queue). Weights are pre-permuted/quantized on the host into
DoubleRow lhsT slot layout [128, 13, 2, 128]. Junk warm-up matmuls ramp the
PE p-state during the input DMA head.

Prior baseline (pure bf16, 9-matmul packing): 130.3 us sim, PE-bound at
122.9 us. This version: PE 88.8 us under a ~95 us DMA floor.
"""

import numpy as np
import ml_dtypes

import concourse.bass as bass
import concourse.mybir as mybir
from concourse import tile

PERMS = np.array(
    [
        [0, 1, 2, 3, 4, 5, 6, 7, 8],
        [3, 0, 1, 6, 4, 2, 7, 8, 5],
        [6, 3, 0, 7, 4, 1, 8, 5, 2],
        [7, 6, 3, 8, 4, 0, 5, 2, 1],
    ],
    dtype=np.int32,
)

N_CORES = 8
B, CIN, COUT, H, W = 16, 64, 64, 128, 128
BPC = B // N_CORES  # batch images per core
HP, WP = H + 2, W + 2  # padded
STRIP = 4  # output rows per PSUM bank fill
NFREE = STRIP * W  # 512

RROWS = 129  # stored tight rows per region (padded rows 0..128)
RSZ = RROWS * W  # elements per region per partition
DTOT = 4 * RSZ  # regions: [A-lo | A-hi | B-hi | B-lo]

NSLOT = 13  # DoubleRow lhsT slots: ap0 M1-M6, shared center, ap1 M1-M6
SLOT_MC = 6
F8 = ml_dtypes.float8_e4m3fn
DR = mybir.MatmulPerfMode.DoubleRow

# tunables (module-level so perf sweeps can override before build_nc)
CHUNKS = [(0, 6), (6, 14)] + [(lo, min(lo + 8, RROWS)) for lo in range(14, RROWS, 8)]
LOOKROWS = 24  # issue a chunk this many output rows before first use
NJUNK = 12
JUNK_COLS = 256
# head DMA issue plan: (queue 's'=sync/HWDGE | 'g'=gpsimd/SWDGE, item)
HEAD_PLAN = [
    ("s", "X0"), ("s", "wt_a"),
    ("g", "wt_b"), ("g", "bias"),
]
# very last strip: ap0 stores whole on LAST_AP0_Q; ap1 is computed/evicted/
# stored in row-pieces (sum = STRIP). One queue char per ap1 piece.
LAST_SPLIT = [4]
LAST_AP0_Q = "s"
LAST_Q = "s"


def _split_multiwait_ctrl(nc, end_times=None):
    """This container's walrus encodes at most one sync-wait per instruction
    (Drain/Matmult/... all hit 'Too many sync wait commands' with >1). Move
    extra waits onto single-wait NOPs preceding the instruction on the same
    engine.

    Multi-waits are ordered so the latest-completing sem stays on the real
    instruction: earlier NoOp waits then retire during its stall window
    instead of serializing after it. Completion-time key: `end_times` (a
    {instruction_name: simulated end ns} map from a prior TimelineSim pass)
    when given, else the program position of the sem's last updater.
    """
    nsplit = 0
    for f in nc.m.functions:
        upd = {}
        order = {}
        idx = 0
        for blk in f.blocks:
            for inst in blk.instructions:
                idx += 1
                order[inst.name] = idx
                s2 = inst.sync_info
                if s2 is not None:
                    for u in s2.on_update:
                        upd.setdefault(u.ant_name, []).append((idx, inst.name))

        def sort_key(iname):
            def k(w):
                us = upd.get(w.ant_name, [])
                if not us:
                    return 0.0
                if end_times:
                    # relevant updater: last one preceding this instruction
                    my = order.get(iname, 1 << 30)
                    prev = [n for (i, n) in us if i < my]
                    target = prev[-1] if prev else us[-1][1]
                    et = end_times.get(target)
                    if et is not None:
                        return et
                return float(us[-1][0])
            return k

        for blk in f.blocks:
            newlist = []
            for inst in blk.instructions:
                si = inst.sync_info
                if si is not None and len(si.on_wait) > 1:
                    waits = list(si.on_wait)
                    if all(w.wait_mode == "sem-ge-imm" for w in waits):
                        # safe to reorder: >= waits are monotonic
                        waits.sort(key=sort_key(inst.name))
                    for w in waits[:-1]:
                        d = mybir.InstNoOp(
                            name=f"{inst.name}-wsplit{nsplit}", ins=[], outs=[]
                        )
                        nsplit += 1
                        d.engine = inst.engine
                        d.sync_info = mybir.SyncInfo(on_wait=[w], on_update=[])
                        newlist.append(d)
                    si.on_wait = [waits[-1]]
                newlist.append(inst)
            blk.instructions = newlist
    return nsplit


def _sim_instruction_end_times(nc):
    """Run TimelineSim capturing each instruction's engine-span end time.
    Used to drive the timing-informed multiwait sort (second build pass)."""
    import concourse.timeline_sim as tsim

    class _Rec:
        def __init__(self):
            self.end = {}
        def enable_explicit_ordering(self, *a, **k): pass
        def reserve_process_order(self, *a, **k): pass
        def add_event(self, process, thread, name, ts, dur=None, unit="s",
                      args=None, clock_name=None, flows=None,
                      terminating_flows=None):
            if args and dur not in (None, "NO_END"):
                n = args.get("instruction_name")
                if n:
                    self.end[n] = max(self.end.get(n, 0.0), ts + dur)
        def add_end(self, *a, **k): pass
        def __getattr__(self, name):
            return lambda *a, **kw: None

    rec = _Rec()
    orig = tsim._build_perfetto
    tsim._build_perfetto = lambda core_id: rec
    try:
        tsim.TimelineSim(nc, trace=True).simulate()
    finally:
        tsim._build_perfetto = orig
    return rec.end


def build_nc(n_batch=BPC, split_ctrl=True, loop_r=None, _end_times=None):
    """loop_r: wrap the whole compute in a For_i repeating it loop_r times —
    used only for on-hardware timing (wall-clock delta between two loop_r
    values divided by the iteration delta isolates per-iteration HW time).

    With split_ctrl, builds twice: the first (position-proxy multiwait sort)
    is simulated to harvest per-instruction end times, which drive a
    timing-informed sort in the second build. Falls back to the proxy build
    if the refinement pass fails for any reason."""
    if split_ctrl and _end_times is None:
        nc = build_nc(n_batch, split_ctrl, loop_r, _end_times={})
        try:
            et = _sim_instruction_end_times(nc)
            if et:
                nc2 = build_nc(n_batch, split_ctrl, loop_r, _end_times=et)
                return nc2
        except Exception:
            pass
        return nc
    f8 = mybir.dt.float8e4
    f32 = mybir.dt.float32
    nc = bass.Bass(target_bir_lowering=False)
    # Strip dead framework preamble work that gates the entry barrier:
    #  - four memsets of const-* scalar tiles this kernel never reads (BIR
    #    verifier: "no reader" for all four)
    #  - the per-engine zero/bcreg/monotonic RegisterMoves: no instruction
    #    in this program references any of those registers (verified by
    #    operand scan; the kernel has no branches/compares/monotonic sems)
    # Dropping them releases the all-engine barrier ~0.7 us earlier.
    import re as _re
    for _f in nc.m.functions:
        for _blk in _f.blocks:
            _blk.instructions = [
                _i for _i in _blk.instructions
                if not (
                    type(_i).__name__ == "InstMemset"
                    and str(_i.engine).endswith("Pool")
                    and _i.outs
                    and "const-" in str(_i.outs[0])
                    and list(map(list, _i.outs[0].ap)) == [[1, 128], [1, 1]]
                )
                and not (
                    type(_i).__name__ == "InstRegisterMove"
                    and _i.outs
                    and _re.search(
                        r"regref='[A-Za-z]+_(bcreg|zero|monotonic)", str(_i.outs[0])
                    )
                )
            ]
    xq_d = nc.declare_dram_parameter(
        "xq", [n_batch, 128, 4, RSZ], f8, isOutput=False
    )
    wt_d = nc.declare_dram_parameter("wt", [128, NSLOT * 2 * 128], f8, isOutput=False)
    bias_d = nc.declare_dram_parameter("bias2", [128, 1], f32, isOutput=False)
    out_d = nc.declare_dram_parameter(
        "out", [4, n_batch, COUT, H, W], mybir.dt.bfloat16, isOutput=True
    )

    with tile.TileContext(nc) as tc:
        with (
            tc.tile_pool(name="const", bufs=1) as const_pool,
            tc.tile_pool(name="xpool", bufs=2) as xpool,
            tc.tile_pool(name="psum", bufs=8, space="PSUM") as psum_pool,
            tc.tile_pool(name="stage", bufs=8) as stage_pool,
        ):
            import contextlib

            loop_ctx = tc.For_i(0, loop_r, 1) if loop_r else contextlib.nullcontext()
            with loop_ctx:
                body(nc, const_pool, xpool, psum_pool, stage_pool,
                     xq_d, wt_d, bias_d, out_d, n_batch,
                     use_swdge=loop_r is None)
    if split_ctrl:
        _split_multiwait_ctrl(nc, end_times=_end_times or None)
    return nc


def body(nc, const_pool, xpool, psum_pool, stage_pool, xq_d, wt_d, bias_d, out_d, n_batch, use_swdge=True):
    gpeng = nc.gpsimd if use_swdge else nc.sync
    bf16 = mybir.dt.bfloat16
    f8 = mybir.dt.float8e4
    f32 = mybir.dt.float32
    GROUP = 2
    if True:
        if True:
            # PE pre-warm: junk matmuls on a zeroed tile ramp the PE p-state
            # while the first x chunk is still in flight. Issued before any
            # DMA so the scheduler gives them the earliest PE priority (a
            # hoisted real Ldweights would head-of-line-block the PE queue
            # on the wt DMA otherwise).
            junk_sb = const_pool.tile([128, max(JUNK_COLS, 128)], bf16)
            nc.vector.memset(junk_sb[:], 0)
            for w in range(NJUNK):
                jps = psum_pool.tile([128, JUNK_COLS], f32, tag="ps", name=f"jps{w}")
                nc.tensor.matmul(jps[:], junk_sb[:, 0:128], junk_sb[:, 0:JUNK_COLS])

            # wt is loaded per the HEAD_PLAN below (split so early matmuls
            # aren't gated on weight slots they don't need yet)
            wt_sb = const_pool.tile([128, NSLOT, 2, 128], f8)
            bias_sb = const_pool.tile([128, 1], f32)

            # fused x tiles [128, 4, RSZ]: four tight-row e4m3 regions
            #   0 = A-lo, 1 = A-hi  (A: partitions [x | x+2cols])
            #   2 = B-hi, 3 = B-lo  (B: partitions [x+1col | x+2rows+1col])
            # (region order makes every subtile pair a small positive
            # stride: M1-3 = {A-lo, A-hi}, M4/MC = {B-hi, B-lo} at stride
            # RSZ; the A->B cross pair M6 at stride RSZ-2W; all < the
            # 32767-element matmul ifmap ISA stride limit)
            xtiles = [
                xpool.tile([128, 4, RSZ], f8, tag="xt", name=f"xt{b}")
                for b in range(n_batch)
            ]

            def load_chunk(b, lo, hi):
                # ALL x loads ride the sync (HWDGE) queue; one DMA per chunk
                # covers rows [lo, hi) of all four regions: per-queue FIFO
                # DGE keeps the serial DMA device in need-order.
                xt = xtiles[b]
                nc.sync.dma_start(
                    xt[:, :, lo * W : hi * W], xq_d[b][:, :, lo * W : hi * W]
                )

            # pending chunk loads, issued interleaved with strips. A chunk
            # (b2, lo, hi) is first needed by strip r0 = lo-5 of image b2
            # (strip windows read A rows <= r0+5, B rows <= r0+4); issue it
            # LOOKROWS of absolute output rows ahead of that so its transfer
            # lands before any PE-queue wait parks on it (in-order SEQ: a
            # late chunk for strip s head-of-line blocks strips < s too).
            pending = [(b, lo, hi) for b in range(n_batch) for (lo, hi) in CHUNKS]
            # head plan: ordered (queue, item) issue list for the first-strip
            # dependencies. Items: wt_a (ap0 slots + shared center), wt_b
            # (ap1 slots), A0/B0 (chunk-0 regions), bias. The DGE pipeline
            # (~625 ns/DMA + 650 ns start latency, serial per queue) paces
            # the head, so order and queue assignment are swept empirically.
            _, lo0, hi0 = pending.pop(0)
            xt0 = xtiles[0]
            nwa = (SLOT_MC + 1) * 2 * 128  # ap0 M1-6 + MC
            items = {
                "wt_a": (wt_sb[:, 0 : SLOT_MC + 1], wt_d[:, 0:nwa]),
                "wt_b": (wt_sb[:, SLOT_MC + 1 :], wt_d[:, nwa:]),
                "X0": (xt0[:, :, lo0 * W : hi0 * W], xq_d[0][:, :, lo0 * W : hi0 * W]),
                "bias": (bias_sb[:], bias_d[:]),
            }
            for q, it in HEAD_PLAN:
                eng = nc.sync if q == "s" else gpeng
                dst, src = items.pop(it)
                eng.dma_start(dst, src)
            assert not items, f"HEAD_PLAN missed {list(items)}"

            def issue_ready(b, r0):
                cur = b * H + r0
                while pending:
                    b2, lo, hi = pending[0]
                    if b2 * H + max(lo - 5, 0) <= cur + LOOKROWS:
                        load_chunk(*pending.pop(0))
                    else:
                        break

            def pair_ap(xt, off, stride, n):
                """Custom DoubleRow rhs: [128, 2(stride), n] reading the
                A-hi region at `off` and `off+stride` (overlapping dims are
                fine for reads). Mutation happens before lowering, so tile
                dep tracking sees the full extent."""
                a = xt[:, 1, off : off + n].unsqueeze(1)
                a.ap[1] = [stride, 2]
                return a

            def emit_center(b, r0, nrows):
                """Shared center-tap DoubleRow matmul + ACT eviction (+bias).
                Returns the f32 center+bias staging tile."""
                xt = xtiles[b]
                nfree = nrows * W
                cps = psum_pool.tile([128, nfree], f32, tag="ps", name=f"cps{b}_{r0}")
                nc.tensor.matmul(
                    cps[:],
                    wt_sb[:, SLOT_MC],
                    xt[:, 2:4, (r0 + 1) * W : (r0 + 1) * W + nfree],
                    perf_mode=DR,
                )
                c2sb = stage_pool.tile([128, nfree], f32, tag="c2", name=f"c2_{b}_{r0}")
                nc.scalar.activation(
                    c2sb[:],
                    cps[:],
                    mybir.ActivationFunctionType.Identity,
                    bias=bias_sb[:],
                )
                return c2sb

            def emit_ap(b, ap, r0, nrows):
                """One angle-pair's 6 DoubleRow matmuls for output rows
                [r0, r0+nrows). For the image's last output row, the
                tap-(2,.)-reading matmuls (M3, M6) shrink by one row: their
                row-129 operand is zero padding (exact for M3/M6-sub0; the
                dropped M6-sub1 Wv_lo term on that single row is ~1e-3).
                Returns the PSUM tile."""
                xt = xtiles[b]
                nfree = nrows * W
                n3 = nfree if r0 + nrows < H else nfree - W
                ps = psum_pool.tile([128, nfree], f32, tag="ps")
                base = 0 if ap == 0 else SLOT_MC + 1
                # M1-M3: A-region tap-pairs (kh,0)|(kh,2), {lo,hi} planes
                for j in range(3):
                    nf = n3 if j == 2 else nfree
                    if nf:
                        nc.tensor.matmul(
                            ps[:, 0:nf],
                            wt_sb[:, base + j],
                            xt[:, 0:2, (r0 + j) * W : (r0 + j) * W + nf],
                            perf_mode=DR,
                            start=(j == 0),
                            stop=False,
                        )
                # M4: B-region vertical pair (0,1)|(2,1), {hi,lo} planes
                nc.tensor.matmul(
                    ps[:],
                    wt_sb[:, base + 3],
                    xt[:, 2:4, r0 * W : r0 * W + nfree],
                    perf_mode=DR,
                    start=False,
                    stop=False,
                )
                # M6: W_lo terms {A_hi rows r0+2, B_hi rows r0}
                if n3:
                    nc.tensor.matmul(
                        ps[:, 0:n3],
                        wt_sb[:, base + 5],
                        pair_ap(xt, (r0 + 2) * W, RSZ - 2 * W, n3),
                        perf_mode=DR,
                        start=False,
                        stop=False,
                    )
                # M5: W_lo terms {A_hi rows r0, rows r0+1} (last: full width)
                nc.tensor.matmul(
                    ps[:],
                    wt_sb[:, base + 4],
                    pair_ap(xt, r0 * W, W, nfree),
                    perf_mode=DR,
                    start=False,
                    stop=True,
                )
                return ps

            def do_strip(b, r0, nrows, sts, st_col):
                """Center + both angle-pairs for output rows [r0, r0+nrows).
                Evictions land at st_col of the per-ap staging tiles."""
                nfree = nrows * W
                c2sb = emit_center(b, r0, nrows)
                for ap in range(2):
                    ps = emit_ap(b, ap, r0, nrows)
                    # eviction: st = ps + (center + bias), DVE only
                    nc.vector.tensor_add(
                        sts[ap][:, st_col : st_col + nfree], ps[:], c2sb[:]
                    )

            n_groups = H // (STRIP * GROUP)
            for b in range(n_batch):
                for g in range(n_groups):
                    rg = g * GROUP * STRIP  # first output row of the group
                    last_group = b == n_batch - 1 and g == n_groups - 1
                    if not last_group:
                        # per-ap staging tiles spanning the whole group
                        # (stored once per ap at group end)
                        sts = [
                            stage_pool.tile(
                                [128, GROUP * NFREE], bf16, tag="st",
                                name=f"st{b}_{g}_{ap}",
                            )
                            for ap in range(2)
                        ]
                        # strips are fully processed one at a time (center,
                        # then both angle-pairs) so a DMA chunk needed by
                        # strip si+1 never head-of-line blocks strip si's
                        # matmuls on the in-order PE queue.
                        for si in range(GROUP):
                            r0 = rg + si * STRIP
                            issue_ready(b, r0)
                            do_strip(b, r0, STRIP, sts, si * NFREE)
                        for ap in range(2):
                            for al in range(2):
                                a = 2 * ap + al
                                eng = nc.sync if al == 0 else gpeng
                                eng.dma_start(
                                    out_d[a, b, :, rg : rg + GROUP * STRIP, :],
                                    sts[ap][al * 64 : (al + 1) * 64, :],
                                )
                    else:
                        # final group: per-strip stores so the first strip's
                        # transfers overlap the last strip's matmuls; the last
                        # strip uses ONE fused two-angle store per ap (single
                        # issue chain ends earlier than staggered transfers)
                        for si in range(GROUP):
                            r0 = rg + si * STRIP
                            issue_ready(b, r0)
                            stk = stage_pool.tile(
                                [128, 2, NFREE], bf16, tag="stz", bufs=2,
                                name=f"stz{si}",
                            )
                            if si < GROUP - 1:
                                stv = [stk[:, ap] for ap in range(2)]
                                do_strip(b, r0, STRIP, stv, 0)
                                for ap in range(2):
                                    for al in range(2):
                                        a = 2 * ap + al
                                        eng = nc.sync if al == 0 else gpeng
                                        eng.dma_start(
                                            out_d[a, b, :, r0 : r0 + STRIP, :],
                                            stk[al * 64 : (al + 1) * 64, ap],
                                        )
                                continue
                            # very last strip: shared center (full strip),
                            # whole ap0, then ap1 in LAST_SPLIT row-pieces
                            q = {"s": nc.sync, "g": gpeng}
                            c2sb = emit_center(b, r0, STRIP)
                            pieces = [(0, STRIP, 0)] + [
                                (sum(LAST_SPLIT[:k]), nr, 1)
                                for k, nr in enumerate(LAST_SPLIT)
                            ]
                            for pi, (ro, nr, ap) in enumerate(pieces):
                                rp = r0 + ro
                                ps = emit_ap(b, ap, rp, nr)
                                sl = stk[:, ap, ro * W : (ro + nr) * W]
                                c2s = c2sb[:, ro * W : (ro + nr) * W]
                                nc.vector.tensor_add(sl, ps[:], c2s)
                                if ap == 0:
                                    eng = q[LAST_AP0_Q]
                                else:
                                    eng = q[LAST_Q[pi - 1] if pi - 1 < len(LAST_Q) else LAST_Q[-1]]
                                eng.dma_start(
                                    out_d[2 * ap : 2 * ap + 2, b, :, rp : rp + nr, :],
                                    sl,
                                )


def _q8(v):
    """e4m3 hi + e4m3 residual planes of v (float32 in, float32 pair out)."""
    hi = v.astype(F8).astype(np.float32)
    lo = (v - hi).astype(F8).astype(np.float32)
    return hi, lo


def prep_weights(weight, bias):
    """wt: [128, 13*2*128] fp8 DoubleRow lhsT slots; bias2: [128, 1] f32.

    Slot layout [128 K, slot, sub, 128 M]: slots 0-5 = ap0 M1-M6, slot 6 =
    shared center, slots 7-12 = ap1 M1-M6. Per ap, with La[t] = [Cin, 2*64]
    the angle-pair's rotated tap-t weights:
      Wp(kh) = [La[3kh] ; La[3kh+2]]  (K = [tap(kh,0) | tap(kh,2)])
      Wv     = [La[1] ; La[7]]        (K = [tap(0,1) | tap(2,1)])
      M1-M3: both subtiles Wp_hi(kh) (x subtiles are {hi, lo} planes)
      M4:    both subtiles Wv_hi
      M5:    {Wp_lo(0), Wp_lo(1)}    (x subtiles {A_hi r0, A_hi r0+1})
      M6:    {Wp_lo(2), Wv_lo}       (x subtiles {A_hi r0+2, B_hi r0})
    Center slot: lower-K = [w4 | w4] M-duplicated hi quantization, upper-K =
    0 (the B-region's upper partitions carry unrelated +2row data); both
    subtiles identical (x subtiles are {hi, lo}); the W4_lo term is dropped
    (~9e-3 total error, gate 2e-2).
    """
    wflat = np.asarray(weight, np.float32).reshape(COUT, CIN, 9)
    # L[t][c, a, o] = wflat[o, c, PERMS[a, t]]
    L = wflat[:, :, PERMS].transpose(3, 1, 2, 0)  # [9, c, a, o]
    wt = np.zeros((128, NSLOT, 2, 128), np.float32)
    for ap in range(2):
        base = 0 if ap == 0 else SLOT_MC + 1
        La = L[:, :, 2 * ap : 2 * ap + 2, :].reshape(9, CIN, 128)  # [t, c, m]
        Wp = [np.concatenate([La[3 * j], La[3 * j + 2]], axis=0) for j in range(3)]
        Wv = np.concatenate([La[1], La[7]], axis=0)
        Wp_q = [_q8(w) for w in Wp]
        Wv_hi, Wv_lo = _q8(Wv)
        for j in range(3):
            wt[:, base + j, 0] = Wp_q[j][0]
            wt[:, base + j, 1] = Wp_q[j][0]
        wt[:, base + 3, 0] = Wv_hi
        wt[:, base + 3, 1] = Wv_hi
        wt[:, base + 4, 0] = Wp_q[0][1]
        wt[:, base + 4, 1] = Wp_q[1][1]
        wt[:, base + 5, 0] = Wp_q[2][1]
        wt[:, base + 5, 1] = Wv_lo
    # shared center: lhsT[c, al*64+o] = W[o, c, 4] duplicated for both angles
    w4 = wflat[:, :, 4].T  # [c, o]
    w4hi = np.concatenate([w4, w4], axis=1).astype(F8).astype(np.float32)
    wt[0:64, SLOT_MC, 0] = w4hi
    wt[0:64, SLOT_MC, 1] = w4hi
    wt8 = wt.reshape(128, NSLOT * 2 * 128).astype(F8)
    bias2 = np.tile(np.asarray(bias, np.float32).reshape(COUT), 2)[:, None]
    return wt8, np.ascontiguousarray(bias2, np.float32)


def prep_x(x):
    """Build the four-region fp8 staging layout on the host.

    Returns xq [nb, 128, 4, RSZ] e4m3, tight rows r = 0..128 of width 128
    (padded row 129 is never read: the last strip's tap-(2,.) matmuls
    shrink instead — that row is zero padding). Regions:
      0 (A-lo): [0:64] lo(xpad[c, r, j]);   [64:128] lo(xpad[c, r, j+2])
      1 (A-hi): [0:64] hi(xpad[c, r, j]);   [64:128] hi(xpad[c, r, j+2])
      2 (B-hi): [0:64] hi(xpad[c, r, j+1]); [64:128] hi(xpad[c, r+2, j+1])
      3 (B-lo): [0:64] lo(xpad[c, r, j+1]); [64:128] lo(xpad[c, r+2, j+1])
    (B upper rows beyond xpad row 130 are zero.)
    """
    nb = x.shape[0]
    xp = np.zeros((nb, CIN, HP + 1, WP), np.float32)  # extra zero row 130
    xp[:, :, 1 : H + 1, 1 : W + 1] = np.asarray(x, np.float32)
    hi, lo = _q8(xp)
    xq = np.zeros((nb, 128, 4, RSZ), F8)
    for pl, ra, rb in ((lo, 0, 3), (hi, 1, 2)):
        q = pl.astype(F8)
        xq[:, 0:64, ra] = q[:, :, 0:RROWS, 0:W].reshape(nb, CIN, RSZ)
        xq[:, 64:128, ra] = q[:, :, 0:RROWS, 2 : 2 + W].reshape(nb, CIN, RSZ)
        xq[:, 0:64, rb] = q[:, :, 0:RROWS, 1 : 1 + W].reshape(nb, CIN, RSZ)
        xq[:, 64:128, rb] = q[:, :, 2 : 2 + RROWS, 1 : 1 + W].reshape(
            nb, CIN, RSZ
        )
    return xq


_CACHE = {}


def _enable_persistent_compile_cache():
    # NEFF compiles take 1-7 minutes; jax's persistent cache serializes the
    # compiled executable (NEFF included) so fresh processes skip the
    # recompile. Best-effort: ignored if the PJRT backend can't serialize.
    try:
        import jax

        jax.config.update("jax_compilation_cache_dir", "/tmp/jax_comp_cache")
        jax.config.update("jax_persistent_cache_min_compile_time_secs", 1.0)
    except Exception:
        pass


def kernel(x, weight, bias):
    from concourse import bass2jax as b2j

    _enable_persistent_compile_cache()

    x = np.asarray(x)
    in_dtype = x.dtype
    xq = prep_x(x)  # [B, 128, 2, DTOT] e4m3
    wt, bias2 = prep_weights(weight, bias)

    if "nc" not in _CACHE:
        _CACHE["nc"] = build_nc()
    nc = _CACHE["nc"]
    in_maps = [
        {"xq": xq[i * BPC : (i + 1) * BPC], "wt": wt, "bias2": bias2}
        for i in range(N_CORES)
    ]
    results = b2j.run_bass_via_pjrt(nc, in_maps, n_cores=N_CORES)
    out = np.stack([r["out"] for r in results])  # [N_CORES, 4, BPC, ...]
    out = out.transpose(1, 0, 2, 3, 4, 5).reshape(4, B, COUT, H, W)
    return out.astype(in_dtype)


# revision 14
# speedup vs baseline: 1.3268x; 1.0172x over previous
"""AdaptiveAngleConv Trainium2 kernel (error-corrected fp8 DoubleRow edition).

Computes, for 4 rotated variants of a 3x3 kernel, y[a] = conv2d(x, rot_a(W)) + b
  x: [16, 64, 128, 128] f32, W: [64, 64, 3, 3] f32, b: [64, 1, 1] f32
  out: [4, 16, 64, 128, 128] f32

Strategy: pure data-parallel over batch (2 images per core, 8 cores, no
collectives). Each core runs an implicit-GEMM conv over 4-row output strips
(N=512 = one f32 PSUM bank), with the 4 angle variants merged into the
matmul M dimension as two angle-pairs (M = 2 angles x 64 Cout = 128).

All matmuls are fp8e4 (e4m3) in DoubleRow perf mode: 2 K=128 subtiles per
matmul (K_eff=256) at 0.5 cycles per output row — 2x bf16 MAC throughput.
Plain e4m3 is far too noisy (4.2% rms per operand), so operands are
error-corrected: both x and W are split into e4m3 hi + e4m3 residual (lo)
planes, and the three significant cross products hi*hi + hi*lo + lo*hi are
accumulated (the dropped lo*lo term is ~2e-3 relative). Per angle-pair per
strip that is exactly 12 K=128 subtile slots = 6 DoubleRow matmuls
(1536 cycles vs 2048+ for bf16):
  M1-M3: pair {tap(kh,0) | tap(kh,2)} (partition-dual A-plane), subtiles
         {x_hi, x_lo} with W_hi               -> hi*hi + hi*lo, kh = 0,1,2
  M4:    pair {tap(0,1) | tap(2,1)} (B-plane), subtiles {x_hi, x_lo} w/ V_hi
  M5:    subtiles {A_hi rows r0, A_hi rows r0+1} with {Wp_lo(0), Wp_lo(1)}
  M6:    subtiles {A_hi rows r0+2, B_hi rows r0} with {Wp_lo(2), Wv_lo}
M5/M6 use custom overlapping access patterns (subtile stride = W or
RSZ-2W within the fused x tile). The rotation-invariant center tap is one
shared DoubleRow matmul (subtiles {x_hi, x_lo} w/ W4_hi; its W4_lo term is
dropped) evicted via ACT (+bias) and added to both angle-pairs on the DVE.
Measured end-to-end error: ~9e-3 (gate 2e-2); PE floor 13 matmuls x 106.7 ns
x 64 strips = 88.8 us.

x is staged in SBUF as ONE fused fp8 tile per image xt[128, 4, 129*128]
with regions [A-lo | A-hi | B-hi | B-lo] (A: tight rows, partitions =
[x | x+2cols]; B: [x+1col | x+2rows+1col]), ordered so every DoubleRow
subtile pair is a positive stride under the 32767-element matmul ifmap ISA
limit, host-prepped in DRAM so each chunk load is ONE DMA. With PE at 88.8 us the
kernel is DMA-device-bound: ~34.2 MB total traffic / 360 GB/s ~ 95 us serial
transfer + DGE head + store tail. All x loads ride the sync/HWDGE queue in
need-order; stores split across HWDGE + SWDGE (see baseline notes: mixing
queues for loads lets late chunks overtake and head-of-line-stall the
in-order PE queue). Weights are pre-permuted/quantized on the host into
DoubleRow lhsT slot layout [128, 13, 2, 128]. Junk warm-up matmuls ramp the
PE p-state during the input DMA head.

Prior baseline (pure bf16, 9-matmul packing): 130.3 us sim, PE-bound at
122.9 us. This version: PE 88.8 us under a ~95 us DMA floor.
"""

import numpy as np
import ml_dtypes

import concourse.bass as bass
import concourse.mybir as mybir
from concourse import tile

PERMS = np.array(
    [
        [0, 1, 2, 3, 4, 5, 6, 7, 8],
        [3, 0, 1, 6, 4, 2, 7, 8, 5],
        [6, 3, 0, 7, 4, 1, 8, 5, 2],
        [7, 6, 3, 8, 4, 0, 5, 2, 1],
    ],
    dtype=np.int32,
)

N_CORES = 8
B, CIN, COUT, H, W = 16, 64, 64, 128, 128
BPC = B // N_CORES  # batch images per core
HP, WP = H + 2, W + 2  # padded
STRIP = 4  # output rows per PSUM bank fill
NFREE = STRIP * W  # 512

RROWS = 129  # stored tight rows per region (padded rows 0..128)
RSZ = RROWS * W  # row elements per region per partition
RSZE = RSZ + 1  # region stride: +1 trailing zero element (the center
#               x-residual read at the very last strip runs one element past
#               the rows; the pad keeps it in-bounds and zero)
DTOT = 3 * RSZE  # regions: [A-hi | A-lo | B-hi]

NSLOT = 13  # DoubleRow lhsT slots: shared center, ap0 M1-M6, ap1 M1-M6
SLOT_MC = 0  # center first: it is the first matmul of the first strip, so
#              the head's first (tiny) weight DMA covers MC+M1 only
F8 = ml_dtypes.float8_e4m3fn
DR = mybir.MatmulPerfMode.DoubleRow

# tunables (module-level so perf sweeps can override before build_nc)
CHUNKS = [(0, 6), (6, 14)] + [(lo, min(lo + 8, RROWS)) for lo in range(14, RROWS, 8)]
LOOKROWS = 24  # issue a chunk this many output rows before first use
NJUNK = 12
JUNK_COLS = 256
# head DMA issue plan: (queue 's'=sync/HWDGE | 'g'=gpsimd/SWDGE, item)
HEAD_PLAN = [
    ("s", "X0"), ("s", "wt_h1"), ("s", "wt_h2"),
    ("g", "wt_b"), ("g", "bias"),
]
# very last strip: ap0 stores whole on LAST_AP0_Q; ap1 is computed/evicted/
# stored in row-pieces (sum = STRIP). One queue char per ap1 piece.
LAST_SPLIT = [4]
LAST_AP0_Q = "s"
LAST_Q = "s"


def _split_multiwait_ctrl(nc, end_times=None):
    """This container's walrus encodes at most one sync-wait per instruction
    (Drain/Matmult/... all hit 'Too many sync wait commands' with >1). Move
    extra waits onto single-wait NOPs preceding the instruction on the same
    engine.

    Multi-waits are ordered so the latest-completing sem stays on the real
    instruction: earlier NoOp waits then retire during its stall window
    instead of serializing after it. Completion-time key: `end_times` (a
    {instruction_name: simulated end ns} map from a prior TimelineSim pass)
    when given, else the program position of the sem's last updater.
    """
    nsplit = 0
    for f in nc.m.functions:
        upd = {}
        order = {}
        idx = 0
        for blk in f.blocks:
            for inst in blk.instructions:
                idx += 1
                order[inst.name] = idx
                s2 = inst.sync_info
                if s2 is not None:
                    for u in s2.on_update:
                        upd.setdefault(u.ant_name, []).append((idx, inst.name))

        def sort_key(iname):
            def k(w):
                us = upd.get(w.ant_name, [])
                if not us:
                    return 0.0
                if end_times:
                    # relevant updater: last one preceding this instruction
                    my = order.get(iname, 1 << 30)
                    prev = [n for (i, n) in us if i < my]
                    target = prev[-1] if prev else us[-1][1]
                    et = end_times.get(target)
                    if et is not None:
                        return et
                return float(us[-1][0])
            return k

        for blk in f.blocks:
            newlist = []
            for inst in blk.instructions:
                si = inst.sync_info
                if si is not None and len(si.on_wait) > 1:
                    waits = list(si.on_wait)
                    if all(w.wait_mode == "sem-ge-imm" for w in waits):
                        # safe to reorder: >= waits are monotonic
                        waits.sort(key=sort_key(inst.name))
                    for w in waits[:-1]:
                        d = mybir.InstNoOp(
                            name=f"{inst.name}-wsplit{nsplit}", ins=[], outs=[]
                        )
                        nsplit += 1
                        d.engine = inst.engine
                        d.sync_info = mybir.SyncInfo(on_wait=[w], on_update=[])
                        newlist.append(d)
                    si.on_wait = [waits[-1]]
                newlist.append(inst)
            blk.instructions = newlist
    return nsplit


def _sim_instruction_end_times(nc):
    """Run TimelineSim capturing each instruction's engine-span end time.
    Used to drive the timing-informed multiwait sort (second build pass)."""
    import concourse.timeline_sim as tsim

    class _Rec:
        def __init__(self):
            self.end = {}
        def enable_explicit_ordering(self, *a, **k): pass
        def reserve_process_order(self, *a, **k): pass
        def add_event(self, process, thread, name, ts, dur=None, unit="s",
                      args=None, clock_name=None, flows=None,
                      terminating_flows=None):
            if args and dur not in (None, "NO_END"):
                n = args.get("instruction_name")
                if n:
                    self.end[n] = max(self.end.get(n, 0.0), ts + dur)
        def add_end(self, *a, **k): pass
        def __getattr__(self, name):
            return lambda *a, **kw: None

    rec = _Rec()
    orig = tsim._build_perfetto
    tsim._build_perfetto = lambda core_id: rec
    try:
        tsim.TimelineSim(nc, trace=True).simulate()
    finally:
        tsim._build_perfetto = orig
    return rec.end


def build_nc(n_batch=BPC, split_ctrl=True, loop_r=None, _end_times=None):
    """loop_r: wrap the whole compute in a For_i repeating it loop_r times —
    used only for on-hardware timing (wall-clock delta between two loop_r
    values divided by the iteration delta isolates per-iteration HW time).

    With split_ctrl, builds twice: the first (position-proxy multiwait sort)
    is simulated to harvest per-instruction end times, which drive a
    timing-informed sort in the second build. Falls back to the proxy build
    if the refinement pass fails for any reason."""
    if split_ctrl and _end_times is None:
        nc = build_nc(n_batch, split_ctrl, loop_r, _end_times={})
        try:
            et = _sim_instruction_end_times(nc)
            if et:
                nc2 = build_nc(n_batch, split_ctrl, loop_r, _end_times=et)
                return nc2
        except Exception:
            pass
        return nc
    f8 = mybir.dt.float8e4
    f32 = mybir.dt.float32
    nc = bass.Bass(target_bir_lowering=False)
    # Strip dead framework preamble work that gates the entry barrier:
    #  - four memsets of const-* scalar tiles this kernel never reads (BIR
    #    verifier: "no reader" for all four)
    #  - the per-engine zero/bcreg/monotonic RegisterMoves: no instruction
    #    in this program references any of those registers (verified by
    #    operand scan; the kernel has no branches/compares/monotonic sems)
    # Dropping them releases the all-engine barrier ~0.7 us earlier.
    import re as _re
    for _f in nc.m.functions:
        for _blk in _f.blocks:
            _blk.instructions = [
                _i for _i in _blk.instructions
                if not (
                    type(_i).__name__ == "InstMemset"
                    and str(_i.engine).endswith("Pool")
                    and _i.outs
                    and "const-" in str(_i.outs[0])
                    and list(map(list, _i.outs[0].ap)) == [[1, 128], [1, 1]]
                )
                and not (
                    type(_i).__name__ == "InstRegisterMove"
                    and _i.outs
                    and _re.search(
                        r"regref='[A-Za-z]+_(bcreg|zero|monotonic)", str(_i.outs[0])
                    )
                )
            ]
    xq_d = nc.declare_dram_parameter(
        "xq", [n_batch, 128, 3, RSZE], f8, isOutput=False
    )
    wt_d = nc.declare_dram_parameter("wt", [128, NSLOT * 2 * 128], f8, isOutput=False)
    bias_d = nc.declare_dram_parameter("bias2", [128, 1], f32, isOutput=False)
    out_d = nc.declare_dram_parameter(
        "out", [4, n_batch, COUT, H, W], mybir.dt.bfloat16, isOutput=True
    )

    nc._pair_fixups = []
    with tile.TileContext(nc) as tc:
        with (
            tc.tile_pool(name="const", bufs=1) as const_pool,
            tc.tile_pool(name="xpool", bufs=2) as xpool,
            tc.tile_pool(name="psum", bufs=8, space="PSUM") as psum_pool,
            tc.tile_pool(name="stage", bufs=8) as stage_pool,
        ):
            import contextlib

            loop_ctx = tc.For_i(0, loop_r, 1) if loop_r else contextlib.nullcontext()
            with loop_ctx:
                body(nc, const_pool, xpool, psum_pool, stage_pool,
                     xq_d, wt_d, bias_d, out_d, n_batch,
                     use_swdge=loop_r is None)
    # apply the long-stride subtile-pair fixups to the POST-lowering
    # physical APs (see dr_pair_matmul)
    fix = dict(nc._pair_fixups)
    nfixed = 0
    for _f in nc.m.functions:
        for _blk in _f.blocks:
            for _i in _blk.instructions:
                st = fix.get(_i.name)
                if st is not None:
                    _i.ins[0].ap[1] = [st, 2]
                    nfixed += 1
    assert nfixed == len(fix), (nfixed, len(fix))
    if split_ctrl:
        _split_multiwait_ctrl(nc, end_times=_end_times or None)
    return nc


def body(nc, const_pool, xpool, psum_pool, stage_pool, xq_d, wt_d, bias_d, out_d, n_batch, use_swdge=True):
    gpeng = nc.gpsimd if use_swdge else nc.sync
    bf16 = mybir.dt.bfloat16
    f8 = mybir.dt.float8e4
    f32 = mybir.dt.float32
    GROUP = 2
    if True:
        if True:
            # PE pre-warm: junk matmuls on a zeroed tile ramp the PE p-state
            # while the first x chunk is still in flight. Issued before any
            # DMA so the scheduler gives them the earliest PE priority (a
            # hoisted real Ldweights would head-of-line-block the PE queue
            # on the wt DMA otherwise).
            junk_sb = const_pool.tile([128, max(JUNK_COLS, 128)], bf16)
            nc.vector.memset(junk_sb[:], 0)
            for w in range(NJUNK):
                jps = psum_pool.tile([128, JUNK_COLS], f32, tag="ps", name=f"jps{w}")
                nc.tensor.matmul(jps[:], junk_sb[:, 0:128], junk_sb[:, 0:JUNK_COLS])

            # wt is loaded per the HEAD_PLAN below (split so early matmuls
            # aren't gated on weight slots they don't need yet)
            wt_sb = const_pool.tile([128, NSLOT, 2, 128], f8)
            bias_sb = const_pool.tile([128, 1], f32)

            # fused x tiles [128, 3, RSZ]: three tight-row e4m3 regions
            #   0 = A-hi, 1 = A-lo  (A: partitions [x | x+2cols])
            #   2 = B-hi            (B: partitions [x+1col | x+2rows+1col])
            # There is NO B-lo region: the vert-(0,1) and center x-residual
            # terms read A-lo at a +1 column offset (tight-row bleed lands
            # on zero padding except a ~1e-3 col-edge term), and the
            # vert-(2,1) x-residual is dropped outright (~9e-3). Region
            # order makes every DoubleRow subtile pair a small positive
            # stride under the 32767-element matmul ifmap ISA limit.
            xtiles = [
                xpool.tile([128, 3, RSZE], f8, tag="xt", name=f"xt{b}")
                for b in range(n_batch)
            ]

            def load_chunk(b, lo, hi):
                # ALL x loads ride the sync (HWDGE) queue; one DMA per chunk
                # covers rows [lo, hi) of all four regions: per-queue FIFO
                # DGE keeps the serial DMA device in need-order.
                xt = xtiles[b]
                he = RSZE if hi >= RROWS else hi * W  # last chunk: +pad elem
                nc.sync.dma_start(
                    xt[:, :, lo * W : he], xq_d[b][:, :, lo * W : he]
                )

            # pending chunk loads, issued interleaved with strips. A chunk
            # (b2, lo, hi) is first needed by strip r0 = lo-5 of image b2
            # (strip windows read A rows <= r0+5, B rows <= r0+4); issue it
            # LOOKROWS of absolute output rows ahead of that so its transfer
            # lands before any PE-queue wait parks on it (in-order SEQ: a
            # late chunk for strip s head-of-line blocks strips < s too).
            pending = [(b, lo, hi) for b in range(n_batch) for (lo, hi) in CHUNKS]
            # head plan: ordered (queue, item) issue list for the first-strip
            # dependencies. Items: wt_a (ap0 slots + shared center), wt_b
            # (ap1 slots), A0/B0 (chunk-0 regions), bias. The DGE pipeline
            # (~625 ns/DMA + 650 ns start latency, serial per queue) paces
            # the head, so order and queue assignment are swept empirically.
            _, lo0, hi0 = pending.pop(0)
            xt0 = xtiles[0]
            SL = 2 * 128  # elements per slot per partition
            items = {
                "wt_h1": (wt_sb[:, 0:2], wt_d[:, 0 : 2 * SL]),  # MC + ap0 M1
                "wt_h2": (wt_sb[:, 2:7], wt_d[:, 2 * SL : 7 * SL]),  # ap0 M2-M6
                "wt_b": (wt_sb[:, 7:], wt_d[:, 7 * SL :]),  # ap1 M1-M6
                "X0": (xt0[:, :, lo0 * W : hi0 * W], xq_d[0][:, :, lo0 * W : hi0 * W]),
                "bias": (bias_sb[:], bias_d[:]),
            }
            for q, it in HEAD_PLAN:
                eng = nc.sync if q == "s" else gpeng
                dst, src = items.pop(it)
                eng.dma_start(dst, src)
            assert not items, f"HEAD_PLAN missed {list(items)}"

            def issue_ready(b, r0):
                cur = b * H + r0
                while pending:
                    b2, lo, hi = pending[0]
                    if b2 * H + max(lo - 5, 0) <= cur + LOOKROWS:
                        load_chunk(*pending.pop(0))
                    else:
                        break

            def pair_ap(xt, ri, off, stride, n):
                """Custom DoubleRow rhs: [128, 2(stride), n] reading
                region `ri` at `off` and `off+stride` (overlapping or
                stride-0 dims are fine for reads)."""
                a = xt[:, ri, off : off + n].unsqueeze(1)
                a.ap[1] = [stride, 2]
                return a

            def dr_pair_matmul(out_ap, wslot, xt, ri, off, stride, n,
                               start, stop):
                """DoubleRow matmul whose rhs subtile pair sits at a LONG
                stride (crossing regions). The tile dep tracker bounds a
                strided dim by its whole span, so a long-stride rhs would
                falsely depend on every x chunk issued so far (pipeline
                lockstep, +170us). Emit with a decoy local stride — the true
                row-chunk dependency is identical because one chunk DMA
                writes all regions' rows and x tiles are write-once — then
                patch the real stride into the already-annotated
                instruction's symbolic AP (lowering reads it afterwards)."""
                a = xt[:, ri, off : off + n].unsqueeze(1)
                a.ap[1] = [1, 2]
                bi = nc.tensor.matmul(
                    out_ap, wslot, a, perf_mode=DR, start=start, stop=stop
                )
                # the TileContext exit pass re-lowers symbolic->physical
                # APs, so record the fixup and apply it to the physical AP
                # after the context closes (build_nc)
                nc._pair_fixups.append((bi.ins.name, stride))
                return bi

            def emit_center(b, r0, nrows):
                """Shared center-tap DoubleRow matmul + ACT eviction (+bias).
                Returns the f32 center+bias staging tile."""
                xt = xtiles[b]
                nfree = nrows * W
                cps = psum_pool.tile([128, nfree], f32, tag="ps", name=f"cps{b}_{r0}")
                # subtiles: {A-lo center col+1 (x residual), B-hi center
                # (x hi)}; both weight subtiles [W4_hi | 0]
                dr_pair_matmul(
                    cps[:], wt_sb[:, SLOT_MC], xt,
                    1, (r0 + 1) * W + 1, RSZE - 1, nfree,
                    start=True, stop=True,
                )
                c2sb = stage_pool.tile([128, nfree], f32, tag="c2", name=f"c2_{b}_{r0}")
                nc.scalar.activation(
                    c2sb[:],
                    cps[:],
                    mybir.ActivationFunctionType.Identity,
                    bias=bias_sb[:],
                )
                return c2sb

            def emit_ap(b, ap, r0, nrows):
                """One angle-pair's 6 DoubleRow matmuls for output rows
                [r0, r0+nrows). For the image's last output row, the
                tap-(2,.)-reading matmuls (M3, M6) shrink by one row: their
                row-129 operand is zero padding (exact for M3/M6-sub0; the
                dropped M6-sub1 Wv_lo term on that single row is ~1e-3).
                Returns the PSUM tile."""
                xt = xtiles[b]
                nfree = nrows * W
                n3 = nfree if r0 + nrows < H else nfree - W
                ps = psum_pool.tile([128, nfree], f32, tag="ps")
                base = 1 if ap == 0 else 7
                # M1-M3: A-region tap-pairs (kh,0)|(kh,2), {hi,lo} planes
                for j in range(3):
                    nf = n3 if j == 2 else nfree
                    if nf:
                        nc.tensor.matmul(
                            ps[:, 0:nf],
                            wt_sb[:, base + j],
                            xt[:, 0:2, (r0 + j) * W : (r0 + j) * W + nf],
                            perf_mode=DR,
                            start=(j == 0),
                            stop=False,
                        )
                # M4: B-hi vertical pair (0,1)|(2,1) x-hi, stride-0
                # subtiles carrying {Wv_hi, Wv_lo}
                nc.tensor.matmul(
                    ps[:],
                    wt_sb[:, base + 3],
                    pair_ap(xt, 2, r0 * W, 0, nfree),
                    perf_mode=DR,
                    start=False,
                    stop=False,
                )
                # M6: {A-hi rows r0+2 w/ Wp_lo(2), A-lo rows r0 col+1 w/
                # [Wv01_hi | 0] (vert-(0,1) x residual)}
                if n3:
                    dr_pair_matmul(
                        ps[:, 0:n3], wt_sb[:, base + 5], xt,
                        0, (r0 + 2) * W, RSZE - 2 * W + 1, n3,
                        start=False, stop=False,
                    )
                # M5: W_lo terms {A_hi rows r0, rows r0+1} (last: full width)
                nc.tensor.matmul(
                    ps[:],
                    wt_sb[:, base + 4],
                    pair_ap(xt, 0, r0 * W, W, nfree),
                    perf_mode=DR,
                    start=False,
                    stop=True,
                )
                return ps

            def do_strip(b, r0, nrows, sts, st_col):
                """Center + both angle-pairs for output rows [r0, r0+nrows).
                Evictions land at st_col of the per-ap staging tiles."""
                nfree = nrows * W
                c2sb = emit_center(b, r0, nrows)
                for ap in range(2):
                    ps = emit_ap(b, ap, r0, nrows)
                    # eviction: st = ps + (center + bias), DVE only
                    nc.vector.tensor_add(
                        sts[ap][:, st_col : st_col + nfree], ps[:], c2sb[:]
                    )

            n_groups = H // (STRIP * GROUP)
            for b in range(n_batch):
                for g in range(n_groups):
                    rg = g * GROUP * STRIP  # first output row of the group
                    last_group = b == n_batch - 1 and g == n_groups - 1
                    if not last_group:
                        # per-ap staging tiles spanning the whole group
                        # (stored once per ap at group end)
                        sts = [
                            stage_pool.tile(
                                [128, GROUP * NFREE], bf16, tag="st",
                                name=f"st{b}_{g}_{ap}",
                            )
                            for ap in range(2)
                        ]
                        # strips are fully processed one at a time (center,
                        # then both angle-pairs) so a DMA chunk needed by
                        # strip si+1 never head-of-line blocks strip si's
                        # matmuls on the in-order PE queue.
                        for si in range(GROUP):
                            r0 = rg + si * STRIP
                            issue_ready(b, r0)
                            do_strip(b, r0, STRIP, sts, si * NFREE)
                        for ap in range(2):
                            for al in range(2):
                                a = 2 * ap + al
                                eng = nc.sync if al == 0 else gpeng
                                eng.dma_start(
                                    out_d[a, b, :, rg : rg + GROUP * STRIP, :],
                                    sts[ap][al * 64 : (al + 1) * 64, :],
                                )
                    else:
                        # final group: per-strip stores so the first strip's
                        # transfers overlap the last strip's matmuls; the last
                        # strip uses ONE fused two-angle store per ap (single
                        # issue chain ends earlier than staggered transfers)
                        for si in range(GROUP):
                            r0 = rg + si * STRIP
                            issue_ready(b, r0)
                            stk = stage_pool.tile(
                                [128, 2, NFREE], bf16, tag="stz", bufs=2,
                                name=f"stz{si}",
                            )
                            if si < GROUP - 1:
                                stv = [stk[:, ap] for ap in range(2)]
                                do_strip(b, r0, STRIP, stv, 0)
                                for ap in range(2):
                                    for al in range(2):
                                        a = 2 * ap + al
                                        eng = nc.sync if al == 0 else gpeng
                                        eng.dma_start(
                                            out_d[a, b, :, r0 : r0 + STRIP, :],
                                            stk[al * 64 : (al + 1) * 64, ap],
                                        )
                                continue
                            # very last strip: shared center (full strip),
                            # whole ap0, then ap1 in LAST_SPLIT row-pieces
                            q = {"s": nc.sync, "g": gpeng}
                            c2sb = emit_center(b, r0, STRIP)
                            pieces = [(0, STRIP, 0)] + [
                                (sum(LAST_SPLIT[:k]), nr, 1)
                                for k, nr in enumerate(LAST_SPLIT)
                            ]
                            for pi, (ro, nr, ap) in enumerate(pieces):
                                rp = r0 + ro
                                ps = emit_ap(b, ap, rp, nr)
                                sl = stk[:, ap, ro * W : (ro + nr) * W]
                                c2s = c2sb[:, ro * W : (ro + nr) * W]
                                nc.vector.tensor_add(sl, ps[:], c2s)
                                if ap == 0:
                                    eng = q[LAST_AP0_Q]
                                else:
                                    eng = q[LAST_Q[pi - 1] if pi - 1 < len(LAST_Q) else LAST_Q[-1]]
                                eng.dma_start(
                                    out_d[2 * ap : 2 * ap + 2, b, :, rp : rp + nr, :],
                                    sl,
                                )


def _q8(v):
    """e4m3 hi + e4m3 residual planes of v (float32 in, float32 pair out)."""
    hi = v.astype(F8).astype(np.float32)
    lo = (v - hi).astype(F8).astype(np.float32)
    return hi, lo


def prep_weights(weight, bias):
    """wt: [128, 13*2*128] fp8 DoubleRow lhsT slots; bias2: [128, 1] f32.

    Slot layout [128 K, slot, sub, 128 M]: slot 0 = shared center, slots
    1-6 = ap0 M1-M6, slots 7-12 = ap1 M1-M6. Per ap, with La[t] = [Cin, 2*64]
    the angle-pair's rotated tap-t weights:
      Wp(kh) = [La[3kh] ; La[3kh+2]]  (K = [tap(kh,0) | tap(kh,2)])
      Wv     = [La[1] ; La[7]]        (K = [tap(0,1) | tap(2,1)])
      M1-M3: both subtiles Wp_hi(kh) (x subtiles are {A-hi, A-lo} planes)
      M4:    {Wv_hi, Wv_lo}          (x subtiles stride-0 on B-hi rows r0)
      M5:    {Wp_lo(0), Wp_lo(1)}    (x subtiles {A-hi r0, A-hi r0+1})
      M6:    {Wp_lo(2), [Wv01_hi|0]} (x: {A-hi r0+2, A-lo r0 col+1})
    Center slot: lower-K = [w4 | w4] M-duplicated hi quantization, upper-K =
    0 (the B-region's upper partitions carry unrelated +2row data); both
    subtiles identical (x subtiles are {hi, lo}); the W4_lo term is dropped
    (~9e-3 total error, gate 2e-2).
    """
    wflat = np.asarray(weight, np.float32).reshape(COUT, CIN, 9)
    # L[t][c, a, o] = wflat[o, c, PERMS[a, t]]
    L = wflat[:, :, PERMS].transpose(3, 1, 2, 0)  # [9, c, a, o]
    wt = np.zeros((128, NSLOT, 2, 128), np.float32)
    for ap in range(2):
        base = 1 if ap == 0 else 7
        La = L[:, :, 2 * ap : 2 * ap + 2, :].reshape(9, CIN, 128)  # [t, c, m]
        Wp = [np.concatenate([La[3 * j], La[3 * j + 2]], axis=0) for j in range(3)]
        Wv = np.concatenate([La[1], La[7]], axis=0)
        Wp_q = [_q8(w) for w in Wp]
        Wv_hi, Wv_lo = _q8(Wv)
        for j in range(3):
            wt[:, base + j, 0] = Wp_q[j][0]
            wt[:, base + j, 1] = Wp_q[j][0]
        wt[:, base + 3, 0] = Wv_hi
        wt[:, base + 3, 1] = Wv_lo
        wt[:, base + 4, 0] = Wp_q[0][1]
        wt[:, base + 4, 1] = Wp_q[1][1]
        wt[:, base + 5, 0] = Wp_q[2][1]
        wt[0:64, base + 5, 1] = Wv_hi[0:64]  # vert-(0,1) hi w/ x-lo; upper K zero
        wt[64:128, base + 5, 1] = 0
    # shared center: lhsT[c, al*64+o] = W[o, c, 4] duplicated for both angles
    w4 = wflat[:, :, 4].T  # [c, o]
    w4hi = np.concatenate([w4, w4], axis=1).astype(F8).astype(np.float32)
    wt[0:64, SLOT_MC, 0] = w4hi
    wt[0:64, SLOT_MC, 1] = w4hi
    wt8 = wt.reshape(128, NSLOT * 2 * 128).astype(F8)
    bias2 = np.tile(np.asarray(bias, np.float32).reshape(COUT), 2)[:, None]
    return wt8, np.ascontiguousarray(bias2, np.float32)


def prep_x(x):
    """Build the three-region fp8 staging layout on the host.

    Returns xq [nb, 128, 3, RSZ] e4m3, tight rows r = 0..128 of width 128
    (padded row 129 is never read: the last strip's tap-(2,.) matmuls
    shrink instead — that row is zero padding). Regions:
      0 (A-hi): [0:64] hi(xpad[c, r, j]);   [64:128] hi(xpad[c, r, j+2])
      1 (A-lo): [0:64] lo(xpad[c, r, j]);   [64:128] lo(xpad[c, r, j+2])
      2 (B-hi): [0:64] hi(xpad[c, r, j+1]); [64:128] hi(xpad[c, r+2, j+1])
    (B upper rows beyond xpad row 130 are zero. There is no B-lo: see the
    module docstring.)
    """
    nb = x.shape[0]
    xp = np.zeros((nb, CIN, HP + 1, WP), np.float32)  # extra zero row 130
    xp[:, :, 1 : H + 1, 1 : W + 1] = np.asarray(x, np.float32)
    hi, lo = _q8(xp)
    xq = np.zeros((nb, 128, 3, RSZE), F8)
    qh = hi.astype(F8)
    ql = lo.astype(F8)
    xq[:, 0:64, 0, :RSZ] = qh[:, :, 0:RROWS, 0:W].reshape(nb, CIN, RSZ)
    xq[:, 64:128, 0, :RSZ] = qh[:, :, 0:RROWS, 2 : 2 + W].reshape(nb, CIN, RSZ)
    xq[:, 0:64, 1, :RSZ] = ql[:, :, 0:RROWS, 0:W].reshape(nb, CIN, RSZ)
    xq[:, 64:128, 1, :RSZ] = ql[:, :, 0:RROWS, 2 : 2 + W].reshape(nb, CIN, RSZ)
    xq[:, 0:64, 2, :RSZ] = qh[:, :, 0:RROWS, 1 : 1 + W].reshape(nb, CIN, RSZ)
    xq[:, 64:128, 2, :RSZ] = qh[:, :, 2 : 2 + RROWS, 1 : 1 + W].reshape(nb, CIN, RSZ)
    return xq


_CACHE = {}


def _enable_persistent_compile_cache():
    # NEFF compiles take 1-7 minutes; jax's persistent cache serializes the
    # compiled executable (NEFF included) so fresh processes skip the
    # recompile. Best-effort: ignored if the PJRT backend can't serialize.
    try:
        import jax

        jax.config.update("jax_compilation_cache_dir", "/tmp/jax_comp_cache")
        jax.config.update("jax_persistent_cache_min_compile_time_secs", 1.0)
    except Exception:
        pass


def kernel(x, weight, bias):
    from concourse import bass2jax as b2j

    _enable_persistent_compile_cache()

    x = np.asarray(x)
    in_dtype = x.dtype
    xq = prep_x(x)  # [B, 128, 2, DTOT] e4m3
    wt, bias2 = prep_weights(weight, bias)

    if "nc" not in _CACHE:
        _CACHE["nc"] = build_nc()
    nc = _CACHE["nc"]
    in_maps = [
        {"xq": xq[i * BPC : (i + 1) * BPC], "wt": wt, "bias2": bias2}
        for i in range(N_CORES)
    ]
    results = b2j.run_bass_via_pjrt(nc, in_maps, n_cores=N_CORES)
    out = np.stack([r["out"] for r in results])  # [N_CORES, 4, BPC, ...]
    out = out.transpose(1, 0, 2, 3, 4, 5).reshape(4, B, COUT, H, W)
    return out.astype(in_dtype)


# revision 18
# speedup vs baseline: 1.3365x; 1.0074x over previous
"""AdaptiveAngleConv Trainium2 kernel (error-corrected fp8 DoubleRow edition).

Computes, for 4 rotated variants of a 3x3 kernel, y[a] = conv2d(x, rot_a(W)) + b
  x: [16, 64, 128, 128] f32, W: [64, 64, 3, 3] f32, b: [64, 1, 1] f32
  out: [4, 16, 64, 128, 128] f32

Strategy: pure data-parallel over batch (2 images per core, 8 cores, no
collectives). Each core runs an implicit-GEMM conv over 4-row output strips
(N=512 = one f32 PSUM bank), with the 4 angle variants merged into the
matmul M dimension as two angle-pairs (M = 2 angles x 64 Cout = 128).

All matmuls are fp8e4 (e4m3) in DoubleRow perf mode: 2 K=128 subtiles per
matmul (K_eff=256) at 0.5 cycles per output row — 2x bf16 MAC throughput.
Plain e4m3 is far too noisy (4.2% rms per operand), so operands are
error-corrected: both x and W are split into e4m3 hi + e4m3 residual (lo)
planes, and the three significant cross products hi*hi + hi*lo + lo*hi are
accumulated (the dropped lo*lo term is ~2e-3 relative). Per angle-pair per
strip that is exactly 12 K=128 subtile slots = 6 DoubleRow matmuls
(1536 cycles vs 2048+ for bf16):
  M1-M3: pair {tap(kh,0) | tap(kh,2)} (partition-dual A-plane), subtiles
         {x_hi, x_lo} with W_hi               -> hi*hi + hi*lo, kh = 0,1,2
  M4:    pair {tap(0,1) | tap(2,1)} (B-plane), subtiles {x_hi, x_lo} w/ V_hi
  M5:    subtiles {A_hi rows r0, A_hi rows r0+1} with {Wp_lo(0), Wp_lo(1)}
  M6:    subtiles {A_hi rows r0+2, B_hi rows r0} with {Wp_lo(2), Wv_lo}
M5/M6 use custom overlapping access patterns (subtile stride = W or
RSZ-2W within the fused x tile). The rotation-invariant center tap is one
shared DoubleRow matmul (subtiles {x_hi, x_lo} w/ W4_hi; its W4_lo term is
dropped) evicted via ACT (+bias) and added to both angle-pairs on the DVE.
Measured end-to-end error: ~9e-3 (gate 2e-2); PE floor 13 matmuls x 106.7 ns
x 64 strips = 88.8 us.

x is staged in SBUF as ONE fused fp8 tile per image xt[128, 4, 129*128]
with regions [A-lo | A-hi | B-hi | B-lo] (A: tight rows, partitions =
[x | x+2cols]; B: [x+1col | x+2rows+1col]), ordered so every DoubleRow
subtile pair is a positive stride under the 32767-element matmul ifmap ISA
limit, host-prepped in DRAM so each chunk load is ONE DMA. With PE at 88.8 us the
kernel is DMA-device-bound: ~34.2 MB total traffic / 360 GB/s ~ 95 us serial
transfer + DGE head + store tail. All x loads ride the sync/HWDGE queue in
need-order; stores split across HWDGE + SWDGE (see baseline notes: mixing
queues for loads lets late chunks overtake and head-of-line-stall the
in-order PE queue). Weights are pre-permuted/quantized on the host into
DoubleRow lhsT slot layout [128, 13, 2, 128]. Junk warm-up matmuls ramp the
PE p-state during the input DMA head.

Prior baseline (pure bf16, 9-matmul packing): 130.3 us sim, PE-bound at
122.9 us. This version: PE 88.8 us under a ~95 us DMA floor.
"""

import numpy as np
import ml_dtypes

import concourse.bass as bass
import concourse.mybir as mybir
from concourse import tile

PERMS = np.array(
    [
        [0, 1, 2, 3, 4, 5, 6, 7, 8],
        [3, 0, 1, 6, 4, 2, 7, 8, 5],
        [6, 3, 0, 7, 4, 1, 8, 5, 2],
        [7, 6, 3, 8, 4, 0, 5, 2, 1],
    ],
    dtype=np.int32,
)

N_CORES = 8
B, CIN, COUT, H, W = 16, 64, 64, 128, 128
BPC = B // N_CORES  # batch images per core
HP, WP = H + 2, W + 2  # padded
STRIP = 4  # output rows per PSUM bank fill
NFREE = STRIP * W  # 512

RROWS = 129  # stored tight rows per region (padded rows 0..128)
RSZ = RROWS * W  # row elements per region per partition
RSZE = RSZ + 1  # region stride: +1 trailing zero element (the center
#               x-residual read at the very last strip runs one element past
#               the rows; the pad keeps it in-bounds and zero)
DTOT = 3 * RSZE  # regions: [A-hi | A-lo | B-hi]

NSLOT = 13  # DoubleRow lhsT slots: shared center, ap0 M1-M6, ap1 M1-M6
SLOT_MC = 0  # center first: it is the first matmul of the first strip, so
#              the head's first (tiny) weight DMA covers MC+M1 only
F8 = ml_dtypes.float8_e4m3fn
DR = mybir.MatmulPerfMode.DoubleRow

# tunables (module-level so perf sweeps can override before build_nc)
CHUNKS = [(0, 6), (6, 10), (10, 14)] + [(lo, min(lo + 8, RROWS)) for lo in range(14, RROWS, 8)]
LOOKROWS = 24  # issue a chunk this many output rows before first use
NJUNK = 12
JUNK_COLS = 256
# head DMA issue plan: (queue 's'=sync/HWDGE | 'g'=gpsimd/SWDGE, item)
HEAD_PLAN = [
    ("s", "X0"), ("s", "wt_h1"), ("s", "wt_h2"),
    ("g", "wt_b"), ("g", "bias"),
]
# very last strip: ap0 stores whole on LAST_AP0_Q; ap1 is computed/evicted/
# stored in row-pieces (sum = STRIP). One queue char per ap1 piece.
LAST_SPLIT = [4]
LAST_AP0_Q = "s"
LAST_Q = "s"
LASTG_SI0_Q = "s"  # queue for the last group's earlier strips' stores
GROUPSTORE_Q = "s"  # queue for the fused whole-group stores
LAST_EVICT_ENG = "vv"  # eviction engine per last-strip piece (v=DVE g=Pool)
# if set, the very last eviction is split DVE [0:h] || Pool [h:], running the
# two halves concurrently so the final store's wait clears sooner
LAST_EVICT_SPLIT = None


def _split_multiwait_ctrl(nc, end_times=None):
    """This container's walrus encodes at most one sync-wait per instruction
    (Drain/Matmult/... all hit 'Too many sync wait commands' with >1). Move
    extra waits onto single-wait NOPs preceding the instruction on the same
    engine.

    Multi-waits are ordered so the latest-completing sem stays on the real
    instruction: earlier NoOp waits then retire during its stall window
    instead of serializing after it. Completion-time key: `end_times` (a
    {instruction_name: simulated end ns} map from a prior TimelineSim pass)
    when given, else the program position of the sem's last updater.
    """
    nsplit = 0
    for f in nc.m.functions:
        upd = {}
        order = {}
        idx = 0
        for blk in f.blocks:
            for inst in blk.instructions:
                idx += 1
                order[inst.name] = idx
                s2 = inst.sync_info
                if s2 is not None:
                    for u in s2.on_update:
                        upd.setdefault(u.ant_name, []).append((idx, inst.name))

        def sort_key(iname):
            def k(w):
                us = upd.get(w.ant_name, [])
                if not us:
                    return 0.0
                if end_times:
                    # relevant updater: last one preceding this instruction
                    my = order.get(iname, 1 << 30)
                    prev = [n for (i, n) in us if i < my]
                    target = prev[-1] if prev else us[-1][1]
                    et = end_times.get(target)
                    if et is not None:
                        return et
                return float(us[-1][0])
            return k

        for blk in f.blocks:
            newlist = []
            for inst in blk.instructions:
                si = inst.sync_info
                if si is not None and len(si.on_wait) > 1:
                    waits = list(si.on_wait)
                    if all(w.wait_mode == "sem-ge-imm" for w in waits):
                        # safe to reorder: >= waits are monotonic
                        waits.sort(key=sort_key(inst.name))
                    for w in waits[:-1]:
                        d = mybir.InstNoOp(
                            name=f"{inst.name}-wsplit{nsplit}", ins=[], outs=[]
                        )
                        nsplit += 1
                        d.engine = inst.engine
                        d.sync_info = mybir.SyncInfo(on_wait=[w], on_update=[])
                        newlist.append(d)
                    si.on_wait = [waits[-1]]
                newlist.append(inst)
            blk.instructions = newlist
    return nsplit


def _sim_instruction_end_times(nc):
    """Run TimelineSim capturing each instruction's engine-span end time.
    Used to drive the timing-informed multiwait sort (second build pass)."""
    import concourse.timeline_sim as tsim

    class _Rec:
        def __init__(self):
            self.end = {}
        def enable_explicit_ordering(self, *a, **k): pass
        def reserve_process_order(self, *a, **k): pass
        def add_event(self, process, thread, name, ts, dur=None, unit="s",
                      args=None, clock_name=None, flows=None,
                      terminating_flows=None):
            if args and dur not in (None, "NO_END"):
                n = args.get("instruction_name")
                if n:
                    self.end[n] = max(self.end.get(n, 0.0), ts + dur)
        def add_end(self, *a, **k): pass
        def __getattr__(self, name):
            return lambda *a, **kw: None

    rec = _Rec()
    orig = tsim._build_perfetto
    tsim._build_perfetto = lambda core_id: rec
    try:
        tsim.TimelineSim(nc, trace=True).simulate()
    finally:
        tsim._build_perfetto = orig
    return rec.end


def build_nc(n_batch=BPC, split_ctrl=True, loop_r=None, _end_times=None):
    """loop_r: wrap the whole compute in a For_i repeating it loop_r times —
    used only for on-hardware timing (wall-clock delta between two loop_r
    values divided by the iteration delta isolates per-iteration HW time).

    With split_ctrl, builds twice: the first (position-proxy multiwait sort)
    is simulated to harvest per-instruction end times, which drive a
    timing-informed sort in the second build. Falls back to the proxy build
    if the refinement pass fails for any reason."""
    if split_ctrl and _end_times is None:
        nc = build_nc(n_batch, split_ctrl, loop_r, _end_times={})
        try:
            et = _sim_instruction_end_times(nc)
            if et:
                nc2 = build_nc(n_batch, split_ctrl, loop_r, _end_times=et)
                return nc2
        except Exception:
            pass
        return nc
    f8 = mybir.dt.float8e4
    f32 = mybir.dt.float32
    nc = bass.Bass(target_bir_lowering=False)
    # Strip dead framework preamble work that gates the entry barrier:
    #  - four memsets of const-* scalar tiles this kernel never reads (BIR
    #    verifier: "no reader" for all four)
    #  - the per-engine zero/bcreg/monotonic RegisterMoves: no instruction
    #    in this program references any of those registers (verified by
    #    operand scan; the kernel has no branches/compares/monotonic sems)
    # Dropping them releases the all-engine barrier ~0.7 us earlier.
    import re as _re
    for _f in nc.m.functions:
        for _blk in _f.blocks:
            _blk.instructions = [
                _i for _i in _blk.instructions
                if not (
                    type(_i).__name__ == "InstMemset"
                    and str(_i.engine).endswith("Pool")
                    and _i.outs
                    and "const-" in str(_i.outs[0])
                    and list(map(list, _i.outs[0].ap)) == [[1, 128], [1, 1]]
                )
                and not (
                    type(_i).__name__ == "InstRegisterMove"
                    and _i.outs
                    and _re.search(
                        r"regref='[A-Za-z]+_(bcreg|zero|monotonic)", str(_i.outs[0])
                    )
                )
            ]
    xq_d = nc.declare_dram_parameter(
        "xq", [n_batch, 128, 3, RSZE], f8, isOutput=False
    )
    wt_d = nc.declare_dram_parameter("wt", [128, NSLOT * 2 * 128], f8, isOutput=False)
    bias_d = nc.declare_dram_parameter("bias2", [128, 1], f32, isOutput=False)
    # partition-major output: out[b, p, ap, h, w] holds angle 2*ap + p//64,
    # channel p%64 — makes a whole 4-angle group store ONE DMA (the angle
    # stride is linear in the partition index)
    out_d = nc.declare_dram_parameter(
        "out", [n_batch, 128, 2, H, W], mybir.dt.bfloat16, isOutput=True
    )

    nc._pair_fixups = []
    with tile.TileContext(nc) as tc:
        with (
            tc.tile_pool(name="const", bufs=1) as const_pool,
            tc.tile_pool(name="xpool", bufs=2) as xpool,
            tc.tile_pool(name="psum", bufs=8, space="PSUM") as psum_pool,
            tc.tile_pool(name="stage", bufs=8) as stage_pool,
        ):
            import contextlib

            loop_ctx = tc.For_i(0, loop_r, 1) if loop_r else contextlib.nullcontext()
            with loop_ctx:
                body(nc, const_pool, xpool, psum_pool, stage_pool,
                     xq_d, wt_d, bias_d, out_d, n_batch,
                     use_swdge=loop_r is None)
    # apply the long-stride subtile-pair fixups to the POST-lowering
    # physical APs (see dr_pair_matmul)
    fix = dict(nc._pair_fixups)
    nfixed = 0
    for _f in nc.m.functions:
        for _blk in _f.blocks:
            for _i in _blk.instructions:
                st = fix.get(_i.name)
                if st is not None:
                    _i.ins[0].ap[1] = [st, 2]
                    nfixed += 1
    assert nfixed == len(fix), (nfixed, len(fix))
    if split_ctrl:
        _split_multiwait_ctrl(nc, end_times=_end_times or None)
    return nc


def body(nc, const_pool, xpool, psum_pool, stage_pool, xq_d, wt_d, bias_d, out_d, n_batch, use_swdge=True):
    gpeng = nc.gpsimd if use_swdge else nc.sync
    bf16 = mybir.dt.bfloat16
    f8 = mybir.dt.float8e4
    f32 = mybir.dt.float32
    GROUP = 2
    if True:
        if True:
            # PE pre-warm: junk matmuls on a zeroed tile ramp the PE p-state
            # while the first x chunk is still in flight. Issued before any
            # DMA so the scheduler gives them the earliest PE priority (a
            # hoisted real Ldweights would head-of-line-block the PE queue
            # on the wt DMA otherwise).
            junk_sb = const_pool.tile([128, max(JUNK_COLS, 128)], bf16)
            nc.vector.memset(junk_sb[:], 0)
            for w in range(NJUNK):
                jps = psum_pool.tile([128, JUNK_COLS], f32, tag="ps", name=f"jps{w}")
                nc.tensor.matmul(jps[:], junk_sb[:, 0:128], junk_sb[:, 0:JUNK_COLS])

            # wt is loaded per the HEAD_PLAN below (split so early matmuls
            # aren't gated on weight slots they don't need yet)
            wt_sb = const_pool.tile([128, NSLOT, 2, 128], f8)
            bias_sb = const_pool.tile([128, 1], f32)

            # fused x tiles [128, 3, RSZ]: three tight-row e4m3 regions
            #   0 = A-hi, 1 = A-lo  (A: partitions [x | x+2cols])
            #   2 = B-hi            (B: partitions [x+1col | x+2rows+1col])
            # There is NO B-lo region: the vert-(0,1) and center x-residual
            # terms read A-lo at a +1 column offset (tight-row bleed lands
            # on zero padding except a ~1e-3 col-edge term), and the
            # vert-(2,1) x-residual is dropped outright (~9e-3). Region
            # order makes every DoubleRow subtile pair a small positive
            # stride under the 32767-element matmul ifmap ISA limit.
            xtiles = [
                xpool.tile([128, 3, RSZE], f8, tag="xt", name=f"xt{b}")
                for b in range(n_batch)
            ]

            def load_chunk(b, lo, hi):
                # ALL x loads ride the sync (HWDGE) queue; one DMA per chunk
                # covers rows [lo, hi) of all four regions: per-queue FIFO
                # DGE keeps the serial DMA device in need-order.
                xt = xtiles[b]
                he = RSZE if hi >= RROWS else hi * W  # last chunk: +pad elem
                nc.sync.dma_start(
                    xt[:, :, lo * W : he], xq_d[b][:, :, lo * W : he]
                )

            # pending chunk loads, issued interleaved with strips. A chunk
            # (b2, lo, hi) is first needed by strip r0 = lo-5 of image b2
            # (strip windows read A rows <= r0+5, B rows <= r0+4); issue it
            # LOOKROWS of absolute output rows ahead of that so its transfer
            # lands before any PE-queue wait parks on it (in-order SEQ: a
            # late chunk for strip s head-of-line blocks strips < s too).
            pending = [(b, lo, hi) for b in range(n_batch) for (lo, hi) in CHUNKS]
            # head plan: ordered (queue, item) issue list for the first-strip
            # dependencies. Items: wt_a (ap0 slots + shared center), wt_b
            # (ap1 slots), A0/B0 (chunk-0 regions), bias. The DGE pipeline
            # (~625 ns/DMA + 650 ns start latency, serial per queue) paces
            # the head, so order and queue assignment are swept empirically.
            _, lo0, hi0 = pending.pop(0)
            xt0 = xtiles[0]
            SL = 2 * 128  # elements per slot per partition
            items = {
                "wt_h1": (wt_sb[:, 0:2], wt_d[:, 0 : 2 * SL]),  # MC + ap0 M1
                "wt_h2": (wt_sb[:, 2:7], wt_d[:, 2 * SL : 7 * SL]),  # ap0 M2-M6
                "wt_b": (wt_sb[:, 7:], wt_d[:, 7 * SL :]),  # ap1 M1-M6
                "X0": (xt0[:, :, lo0 * W : hi0 * W], xq_d[0][:, :, lo0 * W : hi0 * W]),
                "bias": (bias_sb[:], bias_d[:]),
            }
            for q, it in HEAD_PLAN:
                eng = nc.sync if q == "s" else gpeng
                dst, src = items.pop(it)
                eng.dma_start(dst, src)
            assert not items, f"HEAD_PLAN missed {list(items)}"

            def issue_ready(b, r0):
                cur = b * H + r0
                while pending:
                    b2, lo, hi = pending[0]
                    if b2 * H + max(lo - 5, 0) <= cur + LOOKROWS:
                        load_chunk(*pending.pop(0))
                    else:
                        break

            def pair_ap(xt, ri, off, stride, n):
                """Custom DoubleRow rhs: [128, 2(stride), n] reading
                region `ri` at `off` and `off+stride` (overlapping or
                stride-0 dims are fine for reads)."""
                a = xt[:, ri, off : off + n].unsqueeze(1)
                a.ap[1] = [stride, 2]
                return a

            def dr_pair_matmul(out_ap, wslot, xt, ri, off, stride, n,
                               start, stop):
                """DoubleRow matmul whose rhs subtile pair sits at a LONG
                stride (crossing regions). The tile dep tracker bounds a
                strided dim by its whole span, so a long-stride rhs would
                falsely depend on every x chunk issued so far (pipeline
                lockstep, +170us). Emit with a decoy local stride — the true
                row-chunk dependency is identical because one chunk DMA
                writes all regions' rows and x tiles are write-once — then
                patch the real stride into the already-annotated
                instruction's symbolic AP (lowering reads it afterwards)."""
                a = xt[:, ri, off : off + n].unsqueeze(1)
                a.ap[1] = [1, 2]
                bi = nc.tensor.matmul(
                    out_ap, wslot, a, perf_mode=DR, start=start, stop=stop
                )
                # the TileContext exit pass re-lowers symbolic->physical
                # APs, so record the fixup and apply it to the physical AP
                # after the context closes (build_nc)
                nc._pair_fixups.append((bi.ins.name, stride))
                return bi

            def emit_center(b, r0, nrows):
                """Shared center-tap DoubleRow matmul + ACT eviction (+bias).
                Returns the f32 center+bias staging tile."""
                xt = xtiles[b]
                nfree = nrows * W
                cps = psum_pool.tile([128, nfree], f32, tag="ps", name=f"cps{b}_{r0}")
                # subtiles: {A-lo center col+1 (x residual), B-hi center
                # (x hi)}; both weight subtiles [W4_hi | 0]
                dr_pair_matmul(
                    cps[:], wt_sb[:, SLOT_MC], xt,
                    1, (r0 + 1) * W + 1, RSZE - 1, nfree,
                    start=True, stop=True,
                )
                c2sb = stage_pool.tile([128, nfree], f32, tag="c2", name=f"c2_{b}_{r0}")
                nc.scalar.activation(
                    c2sb[:],
                    cps[:],
                    mybir.ActivationFunctionType.Identity,
                    bias=bias_sb[:],
                )
                return c2sb

            def emit_ap(b, ap, r0, nrows):
                """One angle-pair's 6 DoubleRow matmuls for output rows
                [r0, r0+nrows). For the image's last output row, the
                tap-(2,.)-reading matmuls (M3, M6) shrink by one row: their
                row-129 operand is zero padding (exact for M3/M6-sub0; the
                dropped M6-sub1 Wv_lo term on that single row is ~1e-3).
                Returns the PSUM tile."""
                xt = xtiles[b]
                nfree = nrows * W
                n3 = nfree if r0 + nrows < H else nfree - W
                ps = psum_pool.tile([128, nfree], f32, tag="ps")
                base = 1 if ap == 0 else 7
                # M1-M3: A-region tap-pairs (kh,0)|(kh,2), {hi,lo} planes
                for j in range(3):
                    nf = n3 if j == 2 else nfree
                    if nf:
                        nc.tensor.matmul(
                            ps[:, 0:nf],
                            wt_sb[:, base + j],
                            xt[:, 0:2, (r0 + j) * W : (r0 + j) * W + nf],
                            perf_mode=DR,
                            start=(j == 0),
                            stop=False,
                        )
                # M4: B-hi vertical pair (0,1)|(2,1) x-hi, stride-0
                # subtiles carrying {Wv_hi, Wv_lo}
                nc.tensor.matmul(
                    ps[:],
                    wt_sb[:, base + 3],
                    pair_ap(xt, 2, r0 * W, 0, nfree),
                    perf_mode=DR,
                    start=False,
                    stop=False,
                )
                # M6: {A-hi rows r0+2 w/ Wp_lo(2), A-lo rows r0 col+1 w/
                # [Wv01_hi | 0] (vert-(0,1) x residual)}
                if n3:
                    dr_pair_matmul(
                        ps[:, 0:n3], wt_sb[:, base + 5], xt,
                        0, (r0 + 2) * W, RSZE - 2 * W + 1, n3,
                        start=False, stop=False,
                    )
                # M5: W_lo terms {A_hi rows r0, rows r0+1} (last: full width)
                nc.tensor.matmul(
                    ps[:],
                    wt_sb[:, base + 4],
                    pair_ap(xt, 0, r0 * W, W, nfree),
                    perf_mode=DR,
                    start=False,
                    stop=True,
                )
                return ps

            def do_strip(b, r0, nrows, sts, st_col):
                """Center + both angle-pairs for output rows [r0, r0+nrows).
                Evictions land at st_col of the per-ap staging tiles."""
                nfree = nrows * W
                c2sb = emit_center(b, r0, nrows)
                for ap in range(2):
                    ps = emit_ap(b, ap, r0, nrows)
                    # eviction: st = ps + (center + bias), DVE only
                    nc.vector.tensor_add(
                        sts[ap][:, st_col : st_col + nfree], ps[:], c2sb[:]
                    )

            n_groups = H // (STRIP * GROUP)
            for b in range(n_batch):
                for g in range(n_groups):
                    rg = g * GROUP * STRIP  # first output row of the group
                    last_group = b == n_batch - 1 and g == n_groups - 1
                    if not last_group:
                        # fused staging tile spanning the whole group,
                        # stored with ONE 4-angle DMA at group end
                        stg = stage_pool.tile(
                            [128, 2, GROUP * NFREE], bf16, tag="st",
                            name=f"st{b}_{g}",
                        )
                        sts = [stg[:, ap] for ap in range(2)]
                        # strips are fully processed one at a time (center,
                        # then both angle-pairs) so a DMA chunk needed by
                        # strip si+1 never head-of-line blocks strip si's
                        # matmuls on the in-order PE queue.
                        for si in range(GROUP):
                            r0 = rg + si * STRIP
                            issue_ready(b, r0)
                            do_strip(b, r0, STRIP, sts, si * NFREE)
                        eng = nc.sync if GROUPSTORE_Q == "s" else gpeng
                        eng.dma_start(
                            out_d[b, :, :, rg : rg + GROUP * STRIP, :],
                            stg[:],
                        )
                    else:
                        # final group: per-strip stores so the first strip's
                        # transfers overlap the last strip's matmuls; the last
                        # strip uses ONE fused two-angle store per ap (single
                        # issue chain ends earlier than staggered transfers)
                        for si in range(GROUP):
                            r0 = rg + si * STRIP
                            issue_ready(b, r0)
                            stk = stage_pool.tile(
                                [128, 2, NFREE], bf16, tag="stz", bufs=2,
                                name=f"stz{si}",
                            )
                            if si < GROUP - 1:
                                stv = [stk[:, ap] for ap in range(2)]
                                do_strip(b, r0, STRIP, stv, 0)
                                eng = nc.sync if LASTG_SI0_Q[0] == "s" else gpeng
                                eng.dma_start(
                                    out_d[b, :, :, r0 : r0 + STRIP, :],
                                    stk[:],
                                )
                                continue
                            # very last strip: shared center (full strip),
                            # whole ap0, then ap1 in LAST_SPLIT row-pieces
                            q = {"s": nc.sync, "g": gpeng}
                            c2sb = emit_center(b, r0, STRIP)
                            pieces = [(0, STRIP, 0)] + [
                                (sum(LAST_SPLIT[:k]), nr, 1)
                                for k, nr in enumerate(LAST_SPLIT)
                            ]
                            for pi, (ro, nr, ap) in enumerate(pieces):
                                rp = r0 + ro
                                ps = emit_ap(b, ap, rp, nr)
                                sl = stk[:, ap, ro * W : (ro + nr) * W]
                                c2s = c2sb[:, ro * W : (ro + nr) * W]
                                ev = LAST_EVICT_ENG[pi] if pi < len(LAST_EVICT_ENG) else "v"
                                if ev == "g":
                                    nc.gpsimd.tensor_add(sl, ps[:], c2s)
                                elif pi == len(pieces) - 1 and LAST_EVICT_SPLIT:
                                    hsp = LAST_EVICT_SPLIT
                                    nc.vector.tensor_add(
                                        sl[:, 0:hsp], ps[:, 0:hsp], c2s[:, 0:hsp]
                                    )
                                    nc.gpsimd.tensor_add(
                                        sl[:, hsp:], ps[:, hsp:], c2s[:, hsp:]
                                    )
                                else:
                                    nc.vector.tensor_add(sl, ps[:], c2s)
                                if ap == 0:
                                    eng = q[LAST_AP0_Q]
                                else:
                                    eng = q[LAST_Q[pi - 1] if pi - 1 < len(LAST_Q) else LAST_Q[-1]]
                                eng.dma_start(
                                    out_d[b, :, ap, rp : rp + nr, :],
                                    sl,
                                )


def _q8(v):
    """e4m3 hi + e4m3 residual planes of v (float32 in, float32 pair out)."""
    hi = v.astype(F8).astype(np.float32)
    lo = (v - hi).astype(F8).astype(np.float32)
    return hi, lo


def prep_weights(weight, bias):
    """wt: [128, 13*2*128] fp8 DoubleRow lhsT slots; bias2: [128, 1] f32.

    Slot layout [128 K, slot, sub, 128 M]: slot 0 = shared center, slots
    1-6 = ap0 M1-M6, slots 7-12 = ap1 M1-M6. Per ap, with La[t] = [Cin, 2*64]
    the angle-pair's rotated tap-t weights:
      Wp(kh) = [La[3kh] ; La[3kh+2]]  (K = [tap(kh,0) | tap(kh,2)])
      Wv     = [La[1] ; La[7]]        (K = [tap(0,1) | tap(2,1)])
      M1-M3: both subtiles Wp_hi(kh) (x subtiles are {A-hi, A-lo} planes)
      M4:    {Wv_hi, Wv_lo}          (x subtiles stride-0 on B-hi rows r0)
      M5:    {Wp_lo(0), Wp_lo(1)}    (x subtiles {A-hi r0, A-hi r0+1})
      M6:    {Wp_lo(2), [Wv01_hi|0]} (x: {A-hi r0+2, A-lo r0 col+1})
    Center slot: lower-K = [w4 | w4] M-duplicated hi quantization, upper-K =
    0 (the B-region's upper partitions carry unrelated +2row data); both
    subtiles identical (x subtiles are {hi, lo}); the W4_lo term is dropped
    (~9e-3 total error, gate 2e-2).
    """
    wflat = np.asarray(weight, np.float32).reshape(COUT, CIN, 9)
    # L[t][c, a, o] = wflat[o, c, PERMS[a, t]]
    L = wflat[:, :, PERMS].transpose(3, 1, 2, 0)  # [9, c, a, o]
    wt = np.zeros((128, NSLOT, 2, 128), np.float32)
    for ap in range(2):
        base = 1 if ap == 0 else 7
        La = L[:, :, 2 * ap : 2 * ap + 2, :].reshape(9, CIN, 128)  # [t, c, m]
        Wp = [np.concatenate([La[3 * j], La[3 * j + 2]], axis=0) for j in range(3)]
        Wv = np.concatenate([La[1], La[7]], axis=0)
        Wp_q = [_q8(w) for w in Wp]
        Wv_hi, Wv_lo = _q8(Wv)
        for j in range(3):
            wt[:, base + j, 0] = Wp_q[j][0]
            wt[:, base + j, 1] = Wp_q[j][0]
        wt[:, base + 3, 0] = Wv_hi
        wt[:, base + 3, 1] = Wv_lo
        wt[:, base + 4, 0] = Wp_q[0][1]
        wt[:, base + 4, 1] = Wp_q[1][1]
        wt[:, base + 5, 0] = Wp_q[2][1]
        wt[0:64, base + 5, 1] = Wv_hi[0:64]  # vert-(0,1) hi w/ x-lo; upper K zero
        wt[64:128, base + 5, 1] = 0
    # shared center: lhsT[c, al*64+o] = W[o, c, 4] duplicated for both angles
    w4 = wflat[:, :, 4].T  # [c, o]
    w4hi = np.concatenate([w4, w4], axis=1).astype(F8).astype(np.float32)
    wt[0:64, SLOT_MC, 0] = w4hi
    wt[0:64, SLOT_MC, 1] = w4hi
    wt8 = wt.reshape(128, NSLOT * 2 * 128).astype(F8)
    bias2 = np.tile(np.asarray(bias, np.float32).reshape(COUT), 2)[:, None]
    return wt8, np.ascontiguousarray(bias2, np.float32)


def prep_x(x):
    """Build the three-region fp8 staging layout on the host.

    Returns xq [nb, 128, 3, RSZ] e4m3, tight rows r = 0..128 of width 128
    (padded row 129 is never read: the last strip's tap-(2,.) matmuls
    shrink instead — that row is zero padding). Regions:
      0 (A-hi): [0:64] hi(xpad[c, r, j]);   [64:128] hi(xpad[c, r, j+2])
      1 (A-lo): [0:64] lo(xpad[c, r, j]);   [64:128] lo(xpad[c, r, j+2])
      2 (B-hi): [0:64] hi(xpad[c, r, j+1]); [64:128] hi(xpad[c, r+2, j+1])
    (B upper rows beyond xpad row 130 are zero. There is no B-lo: see the
    module docstring.)
    """
    nb = x.shape[0]
    xp = np.zeros((nb, CIN, HP + 1, WP), np.float32)  # extra zero row 130
    xp[:, :, 1 : H + 1, 1 : W + 1] = np.asarray(x, np.float32)
    hi, lo = _q8(xp)
    xq = np.zeros((nb, 128, 3, RSZE), F8)
    qh = hi.astype(F8)
    ql = lo.astype(F8)
    xq[:, 0:64, 0, :RSZ] = qh[:, :, 0:RROWS, 0:W].reshape(nb, CIN, RSZ)
    xq[:, 64:128, 0, :RSZ] = qh[:, :, 0:RROWS, 2 : 2 + W].reshape(nb, CIN, RSZ)
    xq[:, 0:64, 1, :RSZ] = ql[:, :, 0:RROWS, 0:W].reshape(nb, CIN, RSZ)
    xq[:, 64:128, 1, :RSZ] = ql[:, :, 0:RROWS, 2 : 2 + W].reshape(nb, CIN, RSZ)
    xq[:, 0:64, 2, :RSZ] = qh[:, :, 0:RROWS, 1 : 1 + W].reshape(nb, CIN, RSZ)
    xq[:, 64:128, 2, :RSZ] = qh[:, :, 2 : 2 + RROWS, 1 : 1 + W].reshape(nb, CIN, RSZ)
    return xq


_CACHE = {}


def _enable_persistent_compile_cache():
    # NEFF compiles take 1-7 minutes; jax's persistent cache serializes the
    # compiled executable (NEFF included) so fresh processes skip the
    # recompile. Best-effort: ignored if the PJRT backend can't serialize.
    try:
        import jax

        jax.config.update("jax_compilation_cache_dir", "/tmp/jax_comp_cache")
        jax.config.update("jax_persistent_cache_min_compile_time_secs", 1.0)
    except Exception:
        pass


def kernel(x, weight, bias):
    from concourse import bass2jax as b2j

    _enable_persistent_compile_cache()

    x = np.asarray(x)
    in_dtype = x.dtype
    xq = prep_x(x)  # [B, 128, 2, DTOT] e4m3
    wt, bias2 = prep_weights(weight, bias)

    if "nc" not in _CACHE:
        _CACHE["nc"] = build_nc()
    nc = _CACHE["nc"]
    in_maps = [
        {"xq": xq[i * BPC : (i + 1) * BPC], "wt": wt, "bias2": bias2}
        for i in range(N_CORES)
    ]
    results = b2j.run_bass_via_pjrt(nc, in_maps, n_cores=N_CORES)
    out = np.stack([r["out"] for r in results])  # [N_CORES, BPC, 128, 2, H, W]
    out = out.reshape(B, 2, COUT, 2, H, W)  # [b, al, c, ap, h, w]
    out = out.transpose(3, 1, 0, 2, 4, 5).reshape(4, B, COUT, H, W)
    return out.astype(in_dtype)
